# revision 30
# baseline (speedup 1.0000x reference)
"""Trainium2 Bass kernel for nn_DirectDeformGraph (grid-mesh graph build).

Contract: kernel(**inputs) takes the FULL unsharded inputs from
setup_inputs() and returns the full outputs (pts, nrm, radii, lens, areas).

Strategy (hardcoded for H=W=2048, step=2 -> 1024x1024 node grid):
  - Shard the node grid by rows across 8 NeuronCores: 128 node-rows/core.
  - Each core receives only the even pixel rows it needs (its 128 rows plus
    one halo row above and below), does the column-subsample gather
    on-chip, and computes all outputs with pure shift-stencil math --
    the segment sums of the regular 4-edge stencil reduce to shifted adds,
    with the cross-shard (j-1) row handled by a tiny recomputed halo and a
    cyclic-shift matmul on the TensorEngine.
  - Areas use the Lagrange identity |AxB|^2 = |A|^2|B|^2 - (A.B)^2 to reuse
    the squared edge lengths already computed for `lens`.
"""

import os
import sys

import numpy as np

# concourse (Bass) ships with the container; make sure it resolves even when
# kernel.py is invoked from a bare directory.
try:
    import concourse  # noqa: F401
except ImportError:
    for p in ("/opt/trn_rl_repo", "/root/.axon_site/_ro/trn_rl_repo"):
        if os.path.isdir(p) and p not in sys.path:
            sys.path.insert(0, p)
    import concourse  # noqa: F401

NV = 1024          # node grid rows
NU = 1024          # node grid cols
R = 128            # node rows per core
NCORES = 8
F = NU * 3         # 3072 floats per compacted row
FP = F + 24        # padded tile width (pad cols are zeroed)
RAW = NU * 6       # 6144 floats per raw (full) pixel row
W_PIX = 2048

_NC_CACHE = None
LAST_RESULT = None


def _build_nc():
    import concourse.bacc as bacc
    import concourse.mybir as mybir
    from concourse.tile import TileContext

    f32 = mybir.dt.float32
    AF = mybir.ActivationFunctionType
    ALU = mybir.AluOpType

    nc = bacc.Bacc(trn_type="TRN2")

    cand = nc.dram_tensor("cand", [R + 2, RAW], f32, kind="ExternalInput")
    nrmr = nc.dram_tensor("nrmr", [R, RAW], f32, kind="ExternalInput")
    cvec = nc.dram_tensor("cvec", [R, 32], f32, kind="ExternalInput")
    mats = nc.dram_tensor("mats", [R, 256], f32, kind="ExternalInput")

    pts_o = nc.dram_tensor("pts_o", [R, F], f32, kind="ExternalOutput")
    nrm_o = nc.dram_tensor("nrm_o", [R, F], f32, kind="ExternalOutput")
    radii_o = nc.dram_tensor("radii_o", [R, NU], f32, kind="ExternalOutput")
    lens_o = nc.dram_tensor("lens_o", [4, R, NU], f32, kind="ExternalOutput")
    areas_o = nc.dram_tensor("areas_o", [2, R, NU], f32, kind="ExternalOutput")

    def even_cols(t):
        # [p, (i s c)] -> take s=0 (even pixel columns), keep 3 components
        return t.rearrange("p (i s c) -> p i s c", s=2, c=3)[:, :, 0, :]

    def tri(t):
        return t.rearrange("p (i c) -> p i c", c=3)

    with TileContext(nc) as tc:
        with (
            tc.tile_pool(name="pool", bufs=1) as pool,
            tc.tile_pool(name="psum", bufs=1, space="PSUM") as pp,
        ):
            # ---- load raw shards -------------------------------------------------
            # SP ring: halo-below row, rawA left half, P1 row-127 un-spread,
            # spread rows, pts/lens/radii out.
            # Pool ring: rawA right half, P1 shift, norms, nrm/areas out.
            HF = RAW // 2
            HC = F // 2
            rawA = pool.tile([R, RAW], f32, tag="raw24", bufs=2)  # node rows r0..r0+127
            cv = pool.tile([R, 32], f32)
            mm = pool.tile([R, 256], f32)
            # halo-below row via 16-way spread (64 nodes x 6 floats per
            # partition, contiguous chunks -> efficient descriptors)
            hb16 = pool.tile([16, 384], f32)
            nc.sync.dma_start(hb16[:, :],
                              cand[129, :].rearrange("(p x) -> p x", x=384))
            nc.sync.dma_start(rawA[:, 0:HF], cand[1:129, 0:HF])
            nc.gpsimd.dma_start(rawA[:, HF:RAW], cand[1:129, HF:RAW])
            nc.sync.dma_start(cv[:, :], cvec[:, :])
            nc.sync.dma_start(mm[:, :], mats[:, :])
            PHB = pool.tile([16, 192], f32)
            nc.scalar.copy(
                PHB.rearrange("p (i c) -> p i c", c=3),
                hb16.rearrange("p (i s c) -> p i s c", s=2, c=3)[:, :, 0, :],
            )

            # ---- compact even pixel columns (gather) on ACT ----------------------
            P0 = pool.tile([R, FP], f32)
            P1 = pool.tile([R, FP], f32)
            nc.gpsimd.memset(P0[:, F:FP], 0.0)
            nc.gpsimd.memset(P1[:, F:FP], 0.0)
            nc.scalar.copy(tri(P0[:, 0:HC]), even_cols(rawA[:, 0:HF]))
            nc.scalar.copy(tri(P0[:, HC:F]), even_cols(rawA[:, HF:RAW]))
            # P1 = P0 shifted down one row (SBUF->SBUF, no extra HBM read);
            # row 127 = un-spread of the compacted halo-below row.
            nc.sync.dma_start(
                P1[127:128, 0:F].rearrange("p (q k) -> p q k", k=192),
                PHB[:, :],
            )
            nc.gpsimd.dma_start(P1[0:127, 0:HC], P0[1:128, 0:HC])
            nc.gpsimd.dma_start(P1[0:127, HC:F], P0[1:128, HC:F])
            nc.sync.dma_start(pts_o[:, :], P0[:, 0:F])

            def spread_raw_row(drow):
                t = pool.tile([R, 60], f32, tag=f"hs{drow}")
                nc.vector.memset(t[0:1, 0:6], 0.0)
                nc.vector.memset(t[96:128, 54:60], 0.0)
                nc.sync.dma_start(t[0:1, 6:60], cand[drow:drow + 1, 0:54])
                nc.sync.dma_start(
                    t[1:128, 0:48],
                    cand[drow, 42:6138].rearrange("(p x) -> p x", x=48),
                )
                nc.sync.dma_start(
                    t[1:127, 48:60],
                    cand[drow, 90:6138].rearrange("(p x) -> p x", x=48)[:, 0:12],
                )
                nc.sync.dma_start(t[127:128, 48:54], cand[drow:drow + 1, 6138:6144])
                # compact even pixel columns -> [128, 30] (10 nodes x 3)
                c = pool.tile([R, 30], f32, tag=f"hc{drow}")
                nc.scalar.copy(
                    c.rearrange("p (i c) -> p i c", c=3),
                    t.rearrange("p (i s c) -> p i s c", s=2, c=3)[:, :, 0, :],
                )
                return c

            PH = spread_raw_row(0)     # node row r0-1, spread+compacted

            # norms: raw in (Pool), compact, out (Pool)
            nrw = pool.tile([R, RAW], f32, tag="raw24", bufs=2)
            nc.gpsimd.dma_start(nrw[:, 0:HF], nrmr[:, 0:HF])
            nc.gpsimd.dma_start(nrw[:, HF:RAW], nrmr[:, HF:RAW])
            nrc = pool.tile([R, F], f32)
            nc.scalar.copy(tri(nrc[:, 0:HC]), even_cols(nrw[:, 0:HF]))
            nc.scalar.copy(tri(nrc[:, HC:F]), even_cols(nrw[:, HF:RAW]))
            nc.gpsimd.dma_start(nrm_o[:, :], nrc[:, :])

            # spread the r0 row (cand row 1) the same way as the halo rows:
            # PB partition p holds nodes i in [8p-1, 8p+8] (10 nodes x 3).
            PB = spread_raw_row(1)

            # ---- halo W row: W(r0-1, i) = L1(r0-1,i-1) + L2(r0-1,i) + L3(r0-1,i)
            # term1 = |p(r0-1,i-1) - p(r0,i)|, term2 = |p(r0-1,i) - p(r0,i)|,
            # term3 = |p(r0-1,i+1) - p(r0,i)|, each 8 nodes per partition.
            DHL = pool.tile([R, 72], f32)
            nc.vector.tensor_sub(DHL[:, 0:24], PH[:, 0:24], PB[:, 3:27])
            nc.vector.tensor_sub(DHL[:, 24:48], PH[:, 3:27], PB[:, 3:27])
            nc.vector.tensor_sub(DHL[:, 48:72], PH[:, 6:30], PB[:, 3:27])
            SQH = pool.tile([R, 72], f32)
            nc.scalar.square(SQH[:, :], DHL[:, :])
            sqh3 = tri(SQH)
            TH = pool.tile([R, 24], f32)
            SSH = pool.tile([R, 24], f32)
            nc.vector.tensor_add(TH[:, :], sqh3[:, :, 0], sqh3[:, :, 1])
            nc.vector.tensor_add(SSH[:, :], TH[:, :], sqh3[:, :, 2])
            LH = pool.tile([R, 24], f32)
            nc.scalar.activation(LH[:, :], SSH[:, :], AF.Sqrt, scale=cv[:, 5:6])
            # zero the L1 term at i=0 and the L3 term at i=1023 (host-built mask)
            nc.vector.tensor_mul(LH[:, :], LH[:, :], cv[:, 8:32])
            TW = pool.tile([R, 8], f32)
            WH = pool.tile([R, 8], f32)
            nc.vector.tensor_add(TW[:, :], LH[:, 0:8], LH[:, 8:16])
            nc.vector.tensor_add(WH[:, :], TW[:, :], LH[:, 16:24])

            # ---- edge pipeline ---------------------------------------------------
            # Order chosen for overlap: D0 only needs P0 (starts earliest),
            # tri2's dot product runs mid-stream, edge 3 + radii form the tail.
            # L4 holds the four L tensors at stride NU+1 with a leading zero
            # column each, so shifted (i-1) matmul terms read [zero|data] and
            # keep a full-bank out AP.
            NP = NU + 1
            L4 = pool.tile([R, 4 * NP], f32)
            l4v = L4.rearrange("p (e i) -> p e i", i=NP)
            nc.vector.memset(l4v[:, :, 0:1], 0.0)

            D0 = pool.tile([R, F], f32)
            D1 = pool.tile([R, F], f32)
            D2 = pool.tile([R, F], f32)
            nc.vector.tensor_sub(D0[:, 0:HC - 3], P0[:, 0:HC - 3], P0[:, 3:HC])
            nc.vector.tensor_sub(D0[:, HC - 3:F], P0[:, HC - 3:F], P0[:, HC:F + 3])
            nc.vector.tensor_sub(D1[:, :], P0[:, 0:F], P1[:, 3:F + 3])
            nc.vector.tensor_sub(D2[:, :], P0[:, 0:F], P1[:, 0:F])

            SS = {}
            L = {}

            def edge(e, D):
                SQ = pool.tile([R, F], f32, tag="m12", bufs=2, name=f"SQ{e}")
                nc.scalar.square(SQ[:, :], D[:, :])
                s3 = tri(SQ)
                T = pool.tile([R, NU], f32, tag="t4", bufs=3, name=f"T{e}")
                sstag = "t4" if e == 3 else f"ss{e}"
                SSe = pool.tile([R, NU + 1], f32, tag=sstag,
                                bufs=3 if e == 3 else 1, name=f"SS{e}")
                nc.vector.tensor_add(T[:, :], s3[:, :, 0], s3[:, :, 1])
                nc.vector.tensor_add(SSe[:, 0:NU], T[:, :], s3[:, :, 2])
                Le = L4[:, e * NP + 1:(e + 1) * NP]
                if e == 0:
                    nc.scalar.sqrt(Le, SSe[:, 0:NU])
                else:
                    # row mask (j < 1023) folded into the sqrt scale
                    nc.scalar.activation(Le, SSe[:, 0:NU], AF.Sqrt,
                                         scale=cv[:, 0:1])
                if e in (0, 1, 3):
                    # edges leaving i=1023 to the right don't exist
                    nc.gpsimd.memset(Le[:, NU - 1:NU], 0.0)
                nc.sync.dma_start(lens_o[e], Le)
                SS[e] = SSe
                L[e] = Le

            edge(0, D0)
            edge(1, D1)
            edge(2, D2)
            nc.vector.memset(SS[2][:, NU:NU + 1], 0.0)

            # edge 3 (D3 rotates through m12)
            D3 = pool.tile([R, F], f32, tag="m12", bufs=2, name="D3")
            nc.vector.tensor_sub(D3[:, :], P0[:, 3:F + 3], P1[:, 0:F])
            edge(3, D3)

            # tri2 dot product (needs D1, D2 -- runs mid-stream)
            M = pool.tile([R, F], f32, tag="m12", bufs=2)
            nc.vector.tensor_mul(M[:, :], D1[:, :], D2[:, :])
            m3 = tri(M)
            TD = pool.tile([R, NU], f32, tag="t4", bufs=3)
            DOT2 = pool.tile([R, NU], f32)
            nc.vector.tensor_add(TD[:, :], m3[:, :, 0], m3[:, :, 1])
            nc.vector.tensor_add(DOT2[:, :], TD[:, :], m3[:, :, 2])

            # ---- areas via Lagrange identity -------------------------------------
            # |DaxDb|^2 = SSa*SSb - dot^2; for tri1,
            # dot1 = (SS0 + SS1 - |D0-D1|^2)/2 and D0-D1 = -D2(i+1), so
            # dot1 = (SS0 + SS1 - SS2(i+1))/2 -- no product tensor needed.
            def area_tail(t, DOT, qscale, ea, eb):
                Q = pool.tile([R, NU], f32, tag="t4", bufs=3, name=f"Q{t}")
                nc.scalar.activation(Q[:, :], DOT[:, :], AF.Square, scale=qscale)
                TT = pool.tile([R, NU], f32, tag="t4", bufs=3, name=f"TT{t}")
                nc.vector.tensor_mul(TT[:, :], SS[ea][:, 0:NU], SS[eb][:, 0:NU])
                S = pool.tile([R, NU], f32, tag="t4", bufs=3, name=f"S{t}")
                nc.vector.tensor_sub(S[:, :], TT[:, :], Q[:, :])
                nc.vector.tensor_scalar_max(S[:, :], S[:, :], 0.0)
                A = pool.tile([R, NU], f32, tag="t4", bufs=3, name=f"A{t}")
                nc.scalar.activation(A, S[:, :], AF.Sqrt,
                                     scale=cv[:, 3:4], bias=cv[:, 4:5])
                nc.gpsimd.memset(A[:, NU - 1:NU], 0.0)
                nc.gpsimd.dma_start(areas_o[t], A[:, :])

            X = pool.tile([R, NU], f32, tag="t4", bufs=3)
            nc.vector.tensor_add(X[:, :], SS[0][:, 0:NU], SS[1][:, 0:NU])
            DOT1 = pool.tile([R, NU], f32, tag="t4", bufs=3)
            nc.vector.tensor_sub(DOT1[:, :], X[:, :], SS[2][:, 1:NU + 1])
            area_tail(0, DOT1, 0.5, 0, 1)
            area_tail(1, DOT2, 1.0, 1, 2)

            # ---- radii: full stencil on PE (float32r, 1 cyc/row) ------------------
            # ssum = L0+L1+L2 + (L3+L0)(i-1) + shift_down(W),
            # W = L1(i-1)+L2+L3 with the halo row injected at partition 127
            # and rotated in by the cyclic shift matrix SC.
            Wt = pool.tile([R, NU], f32)
            # halo W row lands in partition 127 early (off the critical path);
            # the adds below only touch partitions 0..126 (the rows the cyclic
            # shift consumes alongside row 127).
            nc.gpsimd.dma_start(
                Wt[127:128, :].rearrange("p (q k) -> p q k", k=8), WH[:, :]
            )
            nc.gpsimd.tensor_add(Wt[0:127, :], L[2][0:127, :], L[3][0:127, :])
            nc.gpsimd.tensor_add(Wt[0:127, 1:NU], Wt[0:127, 1:NU],
                                 L[1][0:127, 0:NU - 1])

            rc = pool.tile([R, NU], f32, tag="t4", bufs=3)
            nc.gpsimd.memset(rc[:, :], 0.125)
            nc.gpsimd.memset(rc[:, 0:1], 0.2)
            nc.gpsimd.memset(rc[:, NU - 1:NU], 0.2)
            nc.vector.tensor_scalar(rc[:, :], rc[:, :], cv[:, 1:2], cv[:, 2:3],
                                    ALU.mult, ALU.add)

            # U = L0+L1+L2+(L3+L0)(i-1) on DVE; PE adds shift_down(W) only
            # (longer fp32 matmul accumulation chains crashed on HW)
            U = pool.tile([R, NU], f32)
            nc.gpsimd.tensor_add(U[:, :], L[0], L[1])
            nc.gpsimd.tensor_add(U[:, :], U[:, :], L[2])
            nc.gpsimd.tensor_add(U[:, 1:NU], U[:, 1:NU], L[3][:, 0:NU - 1])
            nc.gpsimd.tensor_add(U[:, 1:NU], U[:, 1:NU], L[0][:, 0:NU - 1])

            ps = pp.tile([R, NU], f32)
            SC = mm[:, 128:256]
            for h in range(2):
                cs = slice(512 * h, 512 * h + 512)
                nc.tensor.matmul(ps[:, cs], SC, Wt[:, cs], start=True,
                                 stop=True)

            # ssum = U + shift_down(W); two DVE ops per half so the radii DMA
            # drains as soon as each bank closes
            RD = pool.tile([R, NU], f32, tag="t4", bufs=3)
            for h in range(2):
                cs = slice(512 * h, 512 * h + 512)
                TMP = pool.tile([R, 512], f32, tag="t4", bufs=3, name=f"TMP{h}")
                nc.vector.tensor_add(TMP[:, :], ps[:, cs], U[:, cs])
                nc.vector.tensor_mul(RD[:, cs], TMP[:, :], rc[:, cs])
                nc.sync.dma_start(radii_o[:, cs], RD[:, cs])

    nc.compile()
    return nc


def _get_nc():
    global _NC_CACHE
    if _NC_CACHE is None:
        _NC_CACHE = _build_nc()
    return _NC_CACHE


def _make_in_maps(candidates, candidates_norms):
    cand2d = np.ascontiguousarray(candidates, dtype=np.float32).reshape(2048, RAW)
    nrm2d = np.ascontiguousarray(candidates_norms, dtype=np.float32).reshape(2048, RAW)

    ident = np.eye(R, dtype=np.float32)
    shift = np.zeros((R, R), dtype=np.float32)
    shift[np.arange(R), (np.arange(R) + 1) % R] = 1.0
    mats = np.concatenate([ident, shift], axis=1)

    in_maps = []
    for c in range(NCORES):
        r0 = c * R
        jrows = np.clip(np.arange(r0 - 1, r0 + R + 1), 0, NV - 1)
        cand_shard = np.ascontiguousarray(cand2d[2 * jrows])
        nrm_shard = np.ascontiguousarray(nrm2d[2 * np.arange(r0, r0 + R)])

        cv = np.zeros((R, 32), dtype=np.float32)
        jglob = np.arange(r0, r0 + R)
        rowmask = (jglob < NV - 1).astype(np.float32)
        cv[:, 0] = rowmask
        rs = np.ones(R, np.float32)
        rb = np.zeros(R, np.float32)
        if c == 0:
            rs[0] = 16.0 / 9.0
            rb[0] = -1.0 / 45.0
        if c == NCORES - 1:
            rs[-1] = 16.0 / 9.0
            rb[-1] = -1.0 / 45.0
        cv[:, 1] = rs
        cv[:, 2] = rb
        cv[:, 3] = 0.25 * rowmask
        cv[:, 4] = np.float32(2.5e-14) * rowmask
        cv[:, 5] = 0.0 if c == 0 else 1.0
        cv[:, 8:32] = 1.0   # LH boundary mask
        cv[0, 8] = 0.0      # L1 halo term invalid at i=0
        cv[127, 31] = 0.0   # L3 halo term invalid at i=1023

        in_maps.append(
            {"cand": cand_shard, "nrmr": nrm_shard, "cvec": cv, "mats": mats}
        )
    return in_maps


def kernel(valid, candidates, candidates_norms, step):
    global LAST_RESULT
    assert int(step) == 2, f"kernel hardcoded for step=2, got {step}"
    assert tuple(np.shape(valid)) == (2048, 2048)
    assert tuple(np.shape(candidates)) == (2048 * 2048, 3)

    from concourse.bass_utils import run_bass_kernel_spmd

    nc = _get_nc()
    in_maps = _make_in_maps(candidates, candidates_norms)
    res = run_bass_kernel_spmd(nc, in_maps, core_ids=list(range(NCORES)))
    LAST_RESULT = res

    N = NV * NU
    pts = np.empty((NV, NU, 3), np.float32)
    nrm = np.empty((NV, NU, 3), np.float32)
    radii = np.empty((NV, NU), np.float32)
    lens = np.empty((4, NV, NU), np.float32)
    areas = np.empty((2, NV, NU), np.float32)
    for c in range(NCORES):
        r0 = c * R
        r = res.results[c]
        pts[r0:r0 + R] = r["pts_o"].reshape(R, NU, 3)
        nrm[r0:r0 + R] = r["nrm_o"].reshape(R, NU, 3)
        radii[r0:r0 + R] = r["radii_o"].reshape(R, NU)
        lens[:, r0:r0 + R] = r["lens_o"].reshape(4, R, NU)
        areas[:, r0:r0 + R] = r["areas_o"].reshape(2, R, NU)

    return (
        pts.reshape(N, 3),
        nrm.reshape(N, 3),
        radii.reshape(N),
        lens.reshape(4 * N),
        areas.reshape(2 * N),
    )


# revision 31
# speedup vs baseline: 1.0526x; 1.0526x over previous
"""Trainium2 Bass kernel for nn_DirectDeformGraph (grid-mesh graph build).

Contract: kernel(**inputs) takes the FULL unsharded inputs from
setup_inputs() and returns the full outputs (pts, nrm, radii, lens, areas).

Strategy (hardcoded for H=W=2048, step=2 -> 1024x1024 node grid):
  - Shard the node grid by rows across 8 NeuronCores: 128 node-rows/core.
  - Each core receives only the even pixel rows it needs (its 128 rows plus
    one halo row above and below), does the column-subsample gather
    on-chip, and computes all outputs with pure shift-stencil math --
    the segment sums of the regular 4-edge stencil reduce to shifted adds,
    with the cross-shard (j-1) row handled by a tiny recomputed halo and a
    cyclic-shift matmul on the TensorEngine.
  - Areas use the Lagrange identity |AxB|^2 = |A|^2|B|^2 - (A.B)^2 to reuse
    the squared edge lengths already computed for `lens`.
"""

import os
import sys

import numpy as np

# concourse (Bass) ships with the container; make sure it resolves even when
# kernel.py is invoked from a bare directory.
try:
    import concourse  # noqa: F401
except ImportError:
    for p in ("/opt/trn_rl_repo", "/root/.axon_site/_ro/trn_rl_repo"):
        if os.path.isdir(p) and p not in sys.path:
            sys.path.insert(0, p)
    import concourse  # noqa: F401

NV = 1024          # node grid rows
NU = 1024          # node grid cols
R = 128            # node rows per core
NCORES = 8
F = NU * 3         # 3072 floats per compacted row
FP = F + 24        # padded tile width (pad cols are zeroed)
RAW = NU * 6       # 6144 floats per raw (full) pixel row
W_PIX = 2048

_NC_CACHE = None
LAST_RESULT = None


def _build_nc():
    import concourse.bacc as bacc
    import concourse.mybir as mybir
    from concourse.tile import TileContext

    f32 = mybir.dt.float32
    AF = mybir.ActivationFunctionType
    ALU = mybir.AluOpType

    nc = bacc.Bacc(trn_type="TRN2")

    cand = nc.dram_tensor("cand", [R + 2, RAW], f32, kind="ExternalInput")
    nrmr = nc.dram_tensor("nrmr", [R, RAW], f32, kind="ExternalInput")
    cvec = nc.dram_tensor("cvec", [R, 32], f32, kind="ExternalInput")
    mats = nc.dram_tensor("mats", [R, 256], f32, kind="ExternalInput")

    pts_o = nc.dram_tensor("pts_o", [R, F], f32, kind="ExternalOutput")
    nrm_o = nc.dram_tensor("nrm_o", [R, F], f32, kind="ExternalOutput")
    radii_o = nc.dram_tensor("radii_o", [R, NU], f32, kind="ExternalOutput")
    lens_o = nc.dram_tensor("lens_o", [4, R, NU], f32, kind="ExternalOutput")
    areas_o = nc.dram_tensor("areas_o", [2, R, NU], f32, kind="ExternalOutput")

    def even_cols(t):
        # [p, (i s c)] -> take s=0 (even pixel columns), keep 3 components
        return t.rearrange("p (i s c) -> p i s c", s=2, c=3)[:, :, 0, :]

    def tri(t):
        return t.rearrange("p (i c) -> p i c", c=3)

    with TileContext(nc) as tc:
        with (
            tc.tile_pool(name="pool", bufs=1) as pool,
            tc.tile_pool(name="psum", bufs=1, space="PSUM") as pp,
        ):
            # ---- load raw shards -------------------------------------------------
            # SP ring: halo-below row, rawA left half, P1 row-127 un-spread,
            # spread rows, pts/lens/radii out.
            # Pool ring: rawA right half, P1 shift, norms, nrm/areas out.
            HF = RAW // 2
            HC = F // 2
            rawA = pool.tile([R, RAW], f32, tag="raw24", bufs=2)  # node rows r0..r0+127
            cv = pool.tile([R, 32], f32)
            mm = pool.tile([R, 256], f32)
            # halo-below row via 16-way spread (64 nodes x 6 floats per
            # partition, contiguous chunks -> efficient descriptors)
            hb16 = pool.tile([16, 384], f32)
            nc.sync.dma_start(hb16[:, :],
                              cand[129, :].rearrange("(p x) -> p x", x=384))
            nc.sync.dma_start(rawA[:, 0:HF], cand[1:129, 0:HF])
            nc.gpsimd.dma_start(rawA[:, HF:RAW], cand[1:129, HF:RAW])
            nc.sync.dma_start(cv[:, :], cvec[:, :])
            nc.sync.dma_start(mm[:, :], mats[:, :])
            # 1/cnt pattern is constant -- build it up front so the radii
            # eviction never waits on it
            rc = pool.tile([R, NU], f32)
            nc.gpsimd.memset(rc[:, :], 0.125)
            nc.gpsimd.memset(rc[:, 0:1], 0.2)
            nc.gpsimd.memset(rc[:, NU - 1:NU], 0.2)
            nc.vector.tensor_scalar(rc[:, :], rc[:, :], cv[:, 1:2], cv[:, 2:3],
                                    ALU.mult, ALU.add)
            PHB = pool.tile([16, 192], f32)
            nc.scalar.copy(
                PHB.rearrange("p (i c) -> p i c", c=3),
                hb16.rearrange("p (i s c) -> p i s c", s=2, c=3)[:, :, 0, :],
            )

            # ---- compact even pixel columns (gather) on ACT ----------------------
            P0 = pool.tile([R, FP], f32)
            P1 = pool.tile([R, FP], f32)
            nc.vector.memset(P0[:, F:FP], 0.0)
            nc.vector.memset(P1[:, F:FP], 0.0)
            nc.scalar.copy(tri(P0[:, 0:HC]), even_cols(rawA[:, 0:HF]))
            nc.scalar.copy(tri(P0[:, HC:F]), even_cols(rawA[:, HF:RAW]))
            # P1 = P0 shifted down one row (SBUF->SBUF, no extra HBM read);
            # row 127 = un-spread of the compacted halo-below row.
            nc.sync.dma_start(
                P1[127:128, 0:F].rearrange("p (q k) -> p q k", k=192),
                PHB[:, :],
            )
            nc.gpsimd.dma_start(P1[0:127, 0:HC], P0[1:128, 0:HC])
            nc.gpsimd.dma_start(P1[0:127, HC:F], P0[1:128, HC:F])
            nc.sync.dma_start(pts_o[:, :], P0[:, 0:F])

            def spread_raw_row(drow):
                t = pool.tile([R, 60], f32, tag=f"hs{drow}")
                nc.vector.memset(t[0:1, 0:6], 0.0)
                nc.vector.memset(t[96:128, 54:60], 0.0)
                nc.sync.dma_start(t[0:1, 6:60], cand[drow:drow + 1, 0:54])
                nc.sync.dma_start(
                    t[1:128, 0:48],
                    cand[drow, 42:6138].rearrange("(p x) -> p x", x=48),
                )
                nc.sync.dma_start(
                    t[1:127, 48:60],
                    cand[drow, 90:6138].rearrange("(p x) -> p x", x=48)[:, 0:12],
                )
                nc.sync.dma_start(t[127:128, 48:54], cand[drow:drow + 1, 6138:6144])
                # compact even pixel columns -> [128, 30] (10 nodes x 3)
                c = pool.tile([R, 30], f32, tag=f"hc{drow}")
                nc.scalar.copy(
                    c.rearrange("p (i c) -> p i c", c=3),
                    t.rearrange("p (i s c) -> p i s c", s=2, c=3)[:, :, 0, :],
                )
                return c

            PH = spread_raw_row(0)     # node row r0-1, spread+compacted

            # norms: raw in (Pool), compact, out (Pool)
            nrw = pool.tile([R, RAW], f32, tag="raw24", bufs=2)
            nc.gpsimd.dma_start(nrw[:, 0:HF], nrmr[:, 0:HF])
            nc.gpsimd.dma_start(nrw[:, HF:RAW], nrmr[:, HF:RAW])
            nrc = pool.tile([R, F], f32)
            nc.scalar.copy(tri(nrc[:, 0:HC]), even_cols(nrw[:, 0:HF]))
            nc.scalar.copy(tri(nrc[:, HC:F]), even_cols(nrw[:, HF:RAW]))
            nc.gpsimd.dma_start(nrm_o[:, :], nrc[:, :])

            # spread the r0 row (cand row 1) the same way as the halo rows:
            # PB partition p holds nodes i in [8p-1, 8p+8] (10 nodes x 3).
            PB = spread_raw_row(1)

            # ---- halo W row: W(r0-1, i) = L1(r0-1,i-1) + L2(r0-1,i) + L3(r0-1,i)
            # term1 = |p(r0-1,i-1) - p(r0,i)|, term2 = |p(r0-1,i) - p(r0,i)|,
            # term3 = |p(r0-1,i+1) - p(r0,i)|, each 8 nodes per partition.
            DHL = pool.tile([R, 72], f32)
            nc.vector.tensor_sub(DHL[:, 0:24], PH[:, 0:24], PB[:, 3:27])
            nc.vector.tensor_sub(DHL[:, 24:48], PH[:, 3:27], PB[:, 3:27])
            nc.vector.tensor_sub(DHL[:, 48:72], PH[:, 6:30], PB[:, 3:27])
            SQH = pool.tile([R, 72], f32)
            nc.scalar.square(SQH[:, :], DHL[:, :])
            sqh3 = tri(SQH)
            TH = pool.tile([R, 24], f32)
            SSH = pool.tile([R, 24], f32)
            nc.vector.tensor_add(TH[:, :], sqh3[:, :, 0], sqh3[:, :, 1])
            nc.vector.tensor_add(SSH[:, :], TH[:, :], sqh3[:, :, 2])
            LH = pool.tile([R, 24], f32)
            nc.scalar.activation(LH[:, :], SSH[:, :], AF.Sqrt, scale=cv[:, 5:6])
            # zero the L1 term at i=0 and the L3 term at i=1023 (host-built mask)
            nc.vector.tensor_mul(LH[:, :], LH[:, :], cv[:, 8:32])
            TW = pool.tile([R, 8], f32)
            WH = pool.tile([R, 8], f32)
            nc.vector.tensor_add(TW[:, :], LH[:, 0:8], LH[:, 8:16])
            nc.vector.tensor_add(WH[:, :], TW[:, :], LH[:, 16:24])

            # ---- edge pipeline ---------------------------------------------------
            # Order chosen for overlap: D0 only needs P0 (starts earliest),
            # tri2's dot product runs mid-stream, edge 3 + radii form the tail.
            # L4 holds the four L tensors at stride NU+1 with a leading zero
            # column each, so shifted (i-1) matmul terms read [zero|data] and
            # keep a full-bank out AP.
            NP = NU + 1
            L4 = pool.tile([R, 4 * NP], f32)
            l4v = L4.rearrange("p (e i) -> p e i", i=NP)
            nc.vector.memset(l4v[:, :, 0:1], 0.0)

            D0 = pool.tile([R, F], f32)
            D1 = pool.tile([R, F], f32)
            D2 = pool.tile([R, F], f32)
            nc.vector.tensor_sub(D0[:, 0:HC - 3], P0[:, 0:HC - 3], P0[:, 3:HC])
            nc.vector.tensor_sub(D0[:, HC - 3:F], P0[:, HC - 3:F], P0[:, HC:F + 3])
            nc.vector.tensor_sub(D1[:, :], P0[:, 0:F], P1[:, 3:F + 3])
            nc.vector.tensor_sub(D2[:, :], P0[:, 0:F], P1[:, 0:F])

            SS = {}
            L = {}

            def edge(e, D):
                SQ = pool.tile([R, F], f32, tag="m12", bufs=2, name=f"SQ{e}")
                nc.scalar.square(SQ[:, :], D[:, :])
                s3 = tri(SQ)
                T = pool.tile([R, NU], f32, tag="t4", bufs=3, name=f"T{e}")
                sstag = "t4" if e == 3 else f"ss{e}"
                SSe = pool.tile([R, NU + 1], f32, tag=sstag,
                                bufs=3 if e == 3 else 1, name=f"SS{e}")
                nc.vector.tensor_add(T[:, :], s3[:, :, 0], s3[:, :, 1])
                nc.vector.tensor_add(SSe[:, 0:NU], T[:, :], s3[:, :, 2])
                Le = L4[:, e * NP + 1:(e + 1) * NP]
                if e == 0:
                    nc.scalar.sqrt(Le, SSe[:, 0:NU])
                else:
                    # row mask (j < 1023) folded into the sqrt scale
                    nc.scalar.activation(Le, SSe[:, 0:NU], AF.Sqrt,
                                         scale=cv[:, 0:1])
                if e in (0, 1, 3):
                    # edges leaving i=1023 to the right don't exist
                    nc.gpsimd.memset(Le[:, NU - 1:NU], 0.0)
                nc.sync.dma_start(lens_o[e], Le)
                SS[e] = SSe
                L[e] = Le

            edge(0, D0)
            edge(1, D1)
            edge(2, D2)
            nc.vector.memset(SS[2][:, NU:NU + 1], 0.0)

            # edge 3 (D3 rotates through m12)
            D3 = pool.tile([R, F], f32, tag="m12", bufs=2, name="D3")
            nc.vector.tensor_sub(D3[:, :], P0[:, 3:F + 3], P1[:, 0:F])
            edge(3, D3)

            # tri2 dot product (needs D1, D2 -- runs mid-stream)
            M = pool.tile([R, F], f32, tag="m12", bufs=2)
            nc.vector.tensor_mul(M[:, :], D1[:, :], D2[:, :])
            m3 = tri(M)
            TD = pool.tile([R, NU], f32, tag="t4", bufs=3)
            DOT2 = pool.tile([R, NU], f32)
            nc.vector.tensor_add(TD[:, :], m3[:, :, 0], m3[:, :, 1])
            nc.vector.tensor_add(DOT2[:, :], TD[:, :], m3[:, :, 2])

            # ---- areas via Lagrange identity -------------------------------------
            # |DaxDb|^2 = SSa*SSb - dot^2; for tri1,
            # dot1 = (SS0 + SS1 - |D0-D1|^2)/2 and D0-D1 = -D2(i+1), so
            # dot1 = (SS0 + SS1 - SS2(i+1))/2 -- no product tensor needed.
            def area_tail(t, DOT, qscale, ea, eb):
                Q = pool.tile([R, NU], f32, tag="t4", bufs=3, name=f"Q{t}")
                nc.scalar.activation(Q[:, :], DOT[:, :], AF.Square, scale=qscale)
                TT = pool.tile([R, NU], f32, tag="t4", bufs=3, name=f"TT{t}")
                nc.gpsimd.tensor_mul(TT[:, :], SS[ea][:, 0:NU], SS[eb][:, 0:NU])
                S = pool.tile([R, NU], f32, tag="t4", bufs=3, name=f"S{t}")
                nc.gpsimd.tensor_sub(S[:, :], TT[:, :], Q[:, :])
                nc.vector.tensor_scalar_max(S[:, :], S[:, :], 0.0)
                A = pool.tile([R, NU], f32, tag="t4", bufs=3, name=f"A{t}")
                nc.scalar.activation(A, S[:, :], AF.Sqrt,
                                     scale=cv[:, 3:4], bias=cv[:, 4:5])
                nc.gpsimd.memset(A[:, NU - 1:NU], 0.0)
                nc.gpsimd.dma_start(areas_o[t], A[:, :])

            X = pool.tile([R, NU], f32, tag="t4", bufs=3)
            nc.vector.tensor_add(X[:, :], SS[0][:, 0:NU], SS[1][:, 0:NU])
            DOT1 = pool.tile([R, NU], f32, tag="t4", bufs=3)
            nc.vector.tensor_sub(DOT1[:, :], X[:, :], SS[2][:, 1:NU + 1])
            area_tail(0, DOT1, 0.5, 0, 1)
            area_tail(1, DOT2, 1.0, 1, 2)

            # ---- radii: full stencil on PE (float32r, 1 cyc/row) ------------------
            # ssum = L0+L1+L2 + (L3+L0)(i-1) + shift_down(W),
            # W = L1(i-1)+L2+L3 with the halo row injected at partition 127
            # and rotated in by the cyclic shift matrix SC.
            Wt = pool.tile([R, NU], f32)
            # halo W row lands in partition 127 early (off the critical path);
            # the adds below only touch partitions 0..126 (the rows the cyclic
            # shift consumes alongside row 127).
            nc.gpsimd.dma_start(
                Wt[127:128, :].rearrange("p (q k) -> p q k", k=8), WH[:, :]
            )
            nc.gpsimd.tensor_add(Wt[0:127, :], L[2][0:127, :], L[3][0:127, :])
            nc.gpsimd.tensor_add(Wt[0:127, 1:NU], Wt[0:127, 1:NU],
                                 L[1][0:127, 0:NU - 1])

            # U = L0+L1+L2+(L3+L0)(i-1) on DVE; PE adds shift_down(W) only
            # (longer fp32 matmul accumulation chains crashed on HW)
            U = pool.tile([R, NU], f32)
            nc.gpsimd.tensor_add(U[:, :], L[0], L[1])
            nc.gpsimd.tensor_add(U[:, :], U[:, :], L[2])
            nc.gpsimd.tensor_add(U[:, 1:NU], U[:, 1:NU], L[3][:, 0:NU - 1])
            nc.gpsimd.tensor_add(U[:, 1:NU], U[:, 1:NU], L[0][:, 0:NU - 1])

            ps = pp.tile([R, NU], f32)
            SC = mm[:, 128:256]
            for h in range(2):
                cs = slice(512 * h, 512 * h + 512)
                nc.tensor.matmul(ps[:, cs], SC, Wt[:, cs], start=True,
                                 stop=True)

            # ssum = U + shift_down(W); two DVE ops per half so the radii DMA
            # drains as soon as each bank closes
            RD = pool.tile([R, NU], f32, tag="t4", bufs=3)
            for h in range(2):
                cs = slice(512 * h, 512 * h + 512)
                TMP = pool.tile([R, 512], f32, tag="t4", bufs=3, name=f"TMP{h}")
                nc.vector.tensor_add(TMP[:, :], ps[:, cs], U[:, cs])
                nc.vector.tensor_mul(RD[:, cs], TMP[:, :], rc[:, cs])
                nc.sync.dma_start(radii_o[:, cs], RD[:, cs])

    nc.compile()
    return nc


def _get_nc():
    global _NC_CACHE
    if _NC_CACHE is None:
        _NC_CACHE = _build_nc()
    return _NC_CACHE


def _make_in_maps(candidates, candidates_norms):
    cand2d = np.ascontiguousarray(candidates, dtype=np.float32).reshape(2048, RAW)
    nrm2d = np.ascontiguousarray(candidates_norms, dtype=np.float32).reshape(2048, RAW)

    ident = np.eye(R, dtype=np.float32)
    shift = np.zeros((R, R), dtype=np.float32)
    shift[np.arange(R), (np.arange(R) + 1) % R] = 1.0
    mats = np.concatenate([ident, shift], axis=1)

    in_maps = []
    for c in range(NCORES):
        r0 = c * R
        jrows = np.clip(np.arange(r0 - 1, r0 + R + 1), 0, NV - 1)
        cand_shard = np.ascontiguousarray(cand2d[2 * jrows])
        nrm_shard = np.ascontiguousarray(nrm2d[2 * np.arange(r0, r0 + R)])

        cv = np.zeros((R, 32), dtype=np.float32)
        jglob = np.arange(r0, r0 + R)
        rowmask = (jglob < NV - 1).astype(np.float32)
        cv[:, 0] = rowmask
        rs = np.ones(R, np.float32)
        rb = np.zeros(R, np.float32)
        if c == 0:
            rs[0] = 16.0 / 9.0
            rb[0] = -1.0 / 45.0
        if c == NCORES - 1:
            rs[-1] = 16.0 / 9.0
            rb[-1] = -1.0 / 45.0
        cv[:, 1] = rs
        cv[:, 2] = rb
        cv[:, 3] = 0.25 * rowmask
        cv[:, 4] = np.float32(2.5e-14) * rowmask
        cv[:, 5] = 0.0 if c == 0 else 1.0
        cv[:, 8:32] = 1.0   # LH boundary mask
        cv[0, 8] = 0.0      # L1 halo term invalid at i=0
        cv[127, 31] = 0.0   # L3 halo term invalid at i=1023

        in_maps.append(
            {"cand": cand_shard, "nrmr": nrm_shard, "cvec": cv, "mats": mats}
        )
    return in_maps


def kernel(valid, candidates, candidates_norms, step):
    global LAST_RESULT
    assert int(step) == 2, f"kernel hardcoded for step=2, got {step}"
    assert tuple(np.shape(valid)) == (2048, 2048)
    assert tuple(np.shape(candidates)) == (2048 * 2048, 3)

    from concourse.bass_utils import run_bass_kernel_spmd

    nc = _get_nc()
    in_maps = _make_in_maps(candidates, candidates_norms)
    res = run_bass_kernel_spmd(nc, in_maps, core_ids=list(range(NCORES)))
    LAST_RESULT = res

    N = NV * NU
    pts = np.empty((NV, NU, 3), np.float32)
    nrm = np.empty((NV, NU, 3), np.float32)
    radii = np.empty((NV, NU), np.float32)
    lens = np.empty((4, NV, NU), np.float32)
    areas = np.empty((2, NV, NU), np.float32)
    for c in range(NCORES):
        r0 = c * R
        r = res.results[c]
        pts[r0:r0 + R] = r["pts_o"].reshape(R, NU, 3)
        nrm[r0:r0 + R] = r["nrm_o"].reshape(R, NU, 3)
        radii[r0:r0 + R] = r["radii_o"].reshape(R, NU)
        lens[:, r0:r0 + R] = r["lens_o"].reshape(4, R, NU)
        areas[:, r0:r0 + R] = r["areas_o"].reshape(2, R, NU)

    return (
        pts.reshape(N, 3),
        nrm.reshape(N, 3),
        radii.reshape(N),
        lens.reshape(4 * N),
        areas.reshape(2 * N),
    )


# revision 34
# speedup vs baseline: 1.0729x; 1.0193x over previous
"""Trainium2 Bass kernel for nn_DirectDeformGraph (grid-mesh graph build).

Contract: kernel(**inputs) takes the FULL unsharded inputs from
setup_inputs() and returns the full outputs (pts, nrm, radii, lens, areas).

Strategy (hardcoded for H=W=2048, step=2 -> 1024x1024 node grid):
  - Shard the node grid by rows across 8 NeuronCores: 128 node-rows/core.
  - Each core receives only the even pixel rows it needs (its 128 rows plus
    one halo row above and below), does the column-subsample gather
    on-chip, and computes all outputs with pure shift-stencil math --
    the segment sums of the regular 4-edge stencil reduce to shifted adds,
    with the cross-shard (j-1) row handled by a tiny recomputed halo and a
    cyclic-shift matmul on the TensorEngine.
  - Areas use the Lagrange identity |AxB|^2 = |A|^2|B|^2 - (A.B)^2 to reuse
    the squared edge lengths already computed for `lens`.
"""

import os
import sys

import numpy as np

# concourse (Bass) ships with the container; make sure it resolves even when
# kernel.py is invoked from a bare directory.
try:
    import concourse  # noqa: F401
except ImportError:
    for p in ("/opt/trn_rl_repo", "/root/.axon_site/_ro/trn_rl_repo"):
        if os.path.isdir(p) and p not in sys.path:
            sys.path.insert(0, p)
    import concourse  # noqa: F401

NV = 1024          # node grid rows
NU = 1024          # node grid cols
R = 128            # node rows per core
NCORES = 8
F = NU * 3         # 3072 floats per compacted row
FP = F + 24        # padded tile width (pad cols are zeroed)
RAW = NU * 6       # 6144 floats per raw (full) pixel row
W_PIX = 2048

_NC_CACHE = None
LAST_RESULT = None


def _build_nc():
    import concourse.bacc as bacc
    import concourse.mybir as mybir
    from concourse.tile import TileContext

    f32 = mybir.dt.float32
    AF = mybir.ActivationFunctionType
    ALU = mybir.AluOpType

    nc = bacc.Bacc(trn_type="TRN2")

    cand = nc.dram_tensor("cand", [R + 2, RAW], f32, kind="ExternalInput")
    nrmr = nc.dram_tensor("nrmr", [R, RAW], f32, kind="ExternalInput")
    cvec = nc.dram_tensor("cvec", [R, 32], f32, kind="ExternalInput")
    mats = nc.dram_tensor("mats", [R, 256], f32, kind="ExternalInput")

    pts_o = nc.dram_tensor("pts_o", [R, F], f32, kind="ExternalOutput")
    nrm_o = nc.dram_tensor("nrm_o", [R, F], f32, kind="ExternalOutput")
    radii_o = nc.dram_tensor("radii_o", [R, NU], f32, kind="ExternalOutput")
    lens_o = nc.dram_tensor("lens_o", [4, R, NU], f32, kind="ExternalOutput")
    areas_o = nc.dram_tensor("areas_o", [2, R, NU], f32, kind="ExternalOutput")

    def even_cols(t):
        # [p, (i s c)] -> take s=0 (even pixel columns), keep 3 components
        return t.rearrange("p (i s c) -> p i s c", s=2, c=3)[:, :, 0, :]

    def tri(t):
        return t.rearrange("p (i c) -> p i c", c=3)

    with TileContext(nc) as tc:
        with (
            tc.tile_pool(name="pool", bufs=1) as pool,
            tc.tile_pool(name="psum", bufs=1, space="PSUM") as pp,
        ):
            # ---- load raw shards -------------------------------------------------
            # SP ring: halo-below row, rawA left half, P1 row-127 un-spread,
            # spread rows, pts/lens/radii out.
            # Pool ring: rawA right half, P1 shift, norms, nrm/areas out.
            HF = RAW // 2
            HC = F // 2
            rawA = pool.tile([R, RAW], f32, tag="raw24", bufs=2)  # node rows r0..r0+127
            cv = pool.tile([R, 32], f32)
            mm = pool.tile([R, 256], f32)
            # halo-below row via 16-way spread (64 nodes x 6 floats per
            # partition, contiguous chunks -> efficient descriptors)
            hb16 = pool.tile([16, 384], f32)
            nc.sync.dma_start(hb16[:, :],
                              cand[129, :].rearrange("(p x) -> p x", x=384))
            nc.sync.dma_start(rawA[:, 0:HF], cand[1:129, 0:HF])
            nc.gpsimd.dma_start(rawA[:, HF:RAW], cand[1:129, HF:RAW])
            nc.sync.dma_start(cv[:, :], cvec[:, :])
            nc.sync.dma_start(mm[:, :], mats[:, :])
            # 1/cnt pattern is constant -- build it up front so the radii
            # eviction never waits on it
            rc = pool.tile([R, NU], f32)
            nc.gpsimd.memset(rc[:, :], 0.125)
            nc.gpsimd.memset(rc[:, 0:1], 0.2)
            nc.gpsimd.memset(rc[:, NU - 1:NU], 0.2)
            nc.vector.tensor_scalar(rc[:, :], rc[:, :], cv[:, 1:2], cv[:, 2:3],
                                    ALU.mult, ALU.add)
            PHB = pool.tile([16, 192], f32)
            nc.scalar.copy(
                PHB.rearrange("p (i c) -> p i c", c=3),
                hb16.rearrange("p (i s c) -> p i s c", s=2, c=3)[:, :, 0, :],
            )

            # ---- compact even pixel columns (gather) on ACT ----------------------
            P0 = pool.tile([R, FP], f32)
            P1 = pool.tile([R, FP], f32)
            nc.vector.memset(P0[:, F:FP], 0.0)
            nc.vector.memset(P1[:, F:FP], 0.0)
            nc.scalar.copy(tri(P0[:, 0:HC]), even_cols(rawA[:, 0:HF]))
            nc.scalar.copy(tri(P0[:, HC:F]), even_cols(rawA[:, HF:RAW]))
            # P1 = P0 shifted down one row (SBUF->SBUF, no extra HBM read);
            # row 127 = un-spread of the compacted halo-below row.
            nc.sync.dma_start(
                P1[127:128, 0:F].rearrange("p (q k) -> p q k", k=192),
                PHB[:, :],
            )
            nc.gpsimd.dma_start(P1[0:127, 0:HC], P0[1:128, 0:HC])
            nc.gpsimd.dma_start(P1[0:127, HC:F], P0[1:128, HC:F])
            nc.sync.dma_start(pts_o[:, :], P0[:, 0:F])

            def spread_raw_row(drow):
                t = pool.tile([R, 60], f32, tag=f"hs{drow}")
                nc.vector.memset(t[0:1, 0:6], 0.0)
                nc.vector.memset(t[96:128, 54:60], 0.0)
                nc.sync.dma_start(t[0:1, 6:60], cand[drow:drow + 1, 0:54])
                nc.sync.dma_start(
                    t[1:128, 0:48],
                    cand[drow, 42:6138].rearrange("(p x) -> p x", x=48),
                )
                nc.sync.dma_start(
                    t[1:127, 48:60],
                    cand[drow, 90:6138].rearrange("(p x) -> p x", x=48)[:, 0:12],
                )
                nc.sync.dma_start(t[127:128, 48:54], cand[drow:drow + 1, 6138:6144])
                # compact even pixel columns -> [128, 30] (10 nodes x 3)
                c = pool.tile([R, 30], f32, tag=f"hc{drow}")
                nc.scalar.copy(
                    c.rearrange("p (i c) -> p i c", c=3),
                    t.rearrange("p (i s c) -> p i s c", s=2, c=3)[:, :, 0, :],
                )
                return c

            PH = spread_raw_row(0)     # node row r0-1, spread+compacted

            # norms: raw in (Pool), compact, out (Pool)
            nrw = pool.tile([R, RAW], f32, tag="raw24", bufs=2)
            nc.gpsimd.dma_start(nrw[:, 0:HF], nrmr[:, 0:HF])
            nc.gpsimd.dma_start(nrw[:, HF:RAW], nrmr[:, HF:RAW])
            nrc = pool.tile([R, F], f32)
            nc.scalar.copy(tri(nrc[:, 0:HC]), even_cols(nrw[:, 0:HF]))
            nc.scalar.copy(tri(nrc[:, HC:F]), even_cols(nrw[:, HF:RAW]))
            nc.gpsimd.dma_start(nrm_o[:, :], nrc[:, :])

            # spread the r0 row (cand row 1) the same way as the halo rows:
            # PB partition p holds nodes i in [8p-1, 8p+8] (10 nodes x 3).
            PB = spread_raw_row(1)

            # ---- halo W row: W(r0-1, i) = L1(r0-1,i-1) + L2(r0-1,i) + L3(r0-1,i)
            # term1 = |p(r0-1,i-1) - p(r0,i)|, term2 = |p(r0-1,i) - p(r0,i)|,
            # term3 = |p(r0-1,i+1) - p(r0,i)|, each 8 nodes per partition.
            DHL = pool.tile([R, 72], f32)
            nc.vector.tensor_sub(DHL[:, 0:24], PH[:, 0:24], PB[:, 3:27])
            nc.vector.tensor_sub(DHL[:, 24:48], PH[:, 3:27], PB[:, 3:27])
            nc.vector.tensor_sub(DHL[:, 48:72], PH[:, 6:30], PB[:, 3:27])
            SQH = pool.tile([R, 72], f32)
            nc.scalar.square(SQH[:, :], DHL[:, :])
            sqh3 = tri(SQH)
            TH = pool.tile([R, 24], f32)
            SSH = pool.tile([R, 24], f32)
            nc.vector.tensor_add(TH[:, :], sqh3[:, :, 0], sqh3[:, :, 1])
            nc.vector.tensor_add(SSH[:, :], TH[:, :], sqh3[:, :, 2])
            LH = pool.tile([R, 24], f32)
            nc.scalar.activation(LH[:, :], SSH[:, :], AF.Sqrt, scale=cv[:, 5:6])
            # zero the L1 term at i=0 and the L3 term at i=1023 (host-built mask)
            nc.vector.tensor_mul(LH[:, :], LH[:, :], cv[:, 8:32])
            TW = pool.tile([R, 8], f32)
            WH = pool.tile([R, 8], f32)
            nc.vector.tensor_add(TW[:, :], LH[:, 0:8], LH[:, 8:16])
            nc.vector.tensor_add(WH[:, :], TW[:, :], LH[:, 16:24])

            # ---- edge pipeline ---------------------------------------------------
            # Order chosen for overlap: D0 only needs P0 (starts earliest),
            # tri2's dot product runs mid-stream, edge 3 + radii form the tail.
            # L4 holds the four L tensors at stride NU+1 with a leading zero
            # column each, so shifted (i-1) matmul terms read [zero|data] and
            # keep a full-bank out AP.
            NP = NU + 1
            L4 = pool.tile([R, 4 * NP], f32)
            l4v = L4.rearrange("p (e i) -> p e i", i=NP)
            nc.vector.memset(l4v[:, :, 0:1], 0.0)

            D0 = pool.tile([R, F], f32)
            D1 = pool.tile([R, F], f32)
            D2 = pool.tile([R, F], f32)
            nc.vector.tensor_sub(D0[:, 0:HC - 3], P0[:, 0:HC - 3], P0[:, 3:HC])
            nc.vector.tensor_sub(D0[:, HC - 3:F], P0[:, HC - 3:F], P0[:, HC:F + 3])
            nc.vector.tensor_sub(D1[:, :], P0[:, 0:F], P1[:, 3:F + 3])
            nc.vector.tensor_sub(D2[:, :], P0[:, 0:F], P1[:, 0:F])

            SS = {}
            L = {}

            def edge(e, D):
                SQ = pool.tile([R, F], f32, tag="m12", bufs=2, name=f"SQ{e}")
                if e == 3:
                    # ACT is the gate at this point in the schedule; square the
                    # last edge on the (idle) GPSIMD instead
                    nc.gpsimd.tensor_mul(SQ[:, :], D[:, :], D[:, :])
                else:
                    nc.scalar.square(SQ[:, :], D[:, :])
                s3 = tri(SQ)
                T = pool.tile([R, NU], f32, tag="t4", bufs=3, name=f"T{e}")
                sstag = "t4" if e == 3 else f"ss{e}"
                SSe = pool.tile([R, NU + 1], f32, tag=sstag,
                                bufs=3 if e == 3 else 1, name=f"SS{e}")
                nc.vector.tensor_add(T[:, :], s3[:, :, 0], s3[:, :, 1])
                nc.vector.tensor_add(SSe[:, 0:NU], T[:, :], s3[:, :, 2])
                Le = L4[:, e * NP + 1:(e + 1) * NP]
                if e == 0:
                    nc.scalar.sqrt(Le, SSe[:, 0:NU])
                else:
                    # row mask (j < 1023) folded into the sqrt scale
                    nc.scalar.activation(Le, SSe[:, 0:NU], AF.Sqrt,
                                         scale=cv[:, 0:1])
                if e in (0, 1, 3):
                    # edges leaving i=1023 to the right don't exist
                    nc.gpsimd.memset(Le[:, NU - 1:NU], 0.0)
                nc.sync.dma_start(lens_o[e], Le)
                SS[e] = SSe
                L[e] = Le

            edge(0, D0)
            edge(1, D1)
            edge(2, D2)
            nc.vector.memset(SS[2][:, NU:NU + 1], 0.0)

            # edge 3 (D3 rotates through m12)
            D3 = pool.tile([R, F], f32, tag="m12", bufs=2, name="D3")
            nc.vector.tensor_sub(D3[:, :], P0[:, 3:F + 3], P1[:, 0:F])
            edge(3, D3)

            # tri2 dot product (needs D1, D2 -- runs mid-stream)
            M = pool.tile([R, F], f32, tag="m12", bufs=2)
            nc.vector.tensor_mul(M[:, :], D1[:, :], D2[:, :])
            m3 = tri(M)
            TD = pool.tile([R, NU], f32, tag="t4", bufs=3)
            DOT2 = pool.tile([R, NU], f32)
            nc.vector.tensor_add(TD[:, :], m3[:, :, 0], m3[:, :, 1])
            nc.vector.tensor_add(DOT2[:, :], TD[:, :], m3[:, :, 2])

            # ---- areas via Lagrange identity -------------------------------------
            # |DaxDb|^2 = SSa*SSb - dot^2; for tri1,
            # dot1 = (SS0 + SS1 - |D0-D1|^2)/2 and D0-D1 = -D2(i+1), so
            # dot1 = (SS0 + SS1 - SS2(i+1))/2 -- no product tensor needed.
            def area_tail(t, DOT, qscale, ea, eb):
                Q = pool.tile([R, NU], f32, tag="t4", bufs=3, name=f"Q{t}")
                nc.scalar.activation(Q[:, :], DOT[:, :], AF.Square, scale=qscale)
                TT = pool.tile([R, NU], f32, tag="t4", bufs=3, name=f"TT{t}")
                nc.gpsimd.tensor_mul(TT[:, :], SS[ea][:, 0:NU], SS[eb][:, 0:NU])
                S = pool.tile([R, NU], f32, tag="t4", bufs=3, name=f"S{t}")
                nc.gpsimd.tensor_sub(S[:, :], TT[:, :], Q[:, :])
                nc.vector.tensor_scalar_max(S[:, :], S[:, :], 0.0)
                A = pool.tile([R, NU], f32, tag="t4", bufs=3, name=f"A{t}")
                nc.scalar.activation(A, S[:, :], AF.Sqrt,
                                     scale=cv[:, 3:4], bias=cv[:, 4:5])
                nc.gpsimd.memset(A[:, NU - 1:NU], 0.0)
                (nc.sync if t == 0 else nc.gpsimd).dma_start(areas_o[t], A[:, :])

            X = pool.tile([R, NU], f32, tag="t4", bufs=3)
            nc.vector.tensor_add(X[:, :], SS[0][:, 0:NU], SS[1][:, 0:NU])
            DOT1 = pool.tile([R, NU], f32, tag="t4", bufs=3)
            nc.vector.tensor_sub(DOT1[:, :], X[:, :], SS[2][:, 1:NU + 1])
            area_tail(0, DOT1, 0.5, 0, 1)
            area_tail(1, DOT2, 1.0, 1, 2)

            # ---- radii: full stencil on PE (float32r, 1 cyc/row) ------------------
            # ssum = L0+L1+L2 + (L3+L0)(i-1) + shift_down(W),
            # W = L1(i-1)+L2+L3 with the halo row injected at partition 127
            # and rotated in by the cyclic shift matrix SC.
            Wt = pool.tile([R, NU], f32)
            # halo W row lands in partition 127 early (off the critical path);
            # the adds below only touch partitions 0..126 (the rows the cyclic
            # shift consumes alongside row 127).
            nc.sync.dma_start(
                Wt[127:128, :].rearrange("p (q k) -> p q k", k=8), WH[:, :]
            )
            nc.gpsimd.tensor_add(Wt[0:127, :], L[2][0:127, :], L[3][0:127, :])
            nc.gpsimd.tensor_add(Wt[0:127, 1:NU], Wt[0:127, 1:NU],
                                 L[1][0:127, 0:NU - 1])

            # U = L0+L1+L2+(L3+L0)(i-1) on DVE; PE adds shift_down(W) only
            # (longer fp32 matmul accumulation chains crashed on HW)
            U = pool.tile([R, NU], f32)
            nc.gpsimd.tensor_add(U[:, :], L[0], L[1])
            nc.gpsimd.tensor_add(U[:, :], U[:, :], L[2])
            nc.gpsimd.tensor_add(U[:, 1:NU], U[:, 1:NU], L[3][:, 0:NU - 1])
            nc.gpsimd.tensor_add(U[:, 1:NU], U[:, 1:NU], L[0][:, 0:NU - 1])

            ps = pp.tile([R, NU], f32)
            SC = mm[:, 128:256]
            for h in range(2):
                cs = slice(512 * h, 512 * h + 512)
                nc.tensor.matmul(ps[:, cs], SC, Wt[:, cs], start=True,
                                 stop=True)

            # ssum = U + shift_down(W); two DVE ops per half so the radii DMA
            # drains as soon as each bank closes
            RD = pool.tile([R, NU], f32, tag="t4", bufs=3)
            for h in range(2):
                cs = slice(512 * h, 512 * h + 512)
                TMP = pool.tile([R, 512], f32, tag="t4", bufs=3, name=f"TMP{h}")
                nc.vector.tensor_add(TMP[:, :], ps[:, cs], U[:, cs])
                nc.vector.tensor_mul(RD[:, cs], TMP[:, :], rc[:, cs])
                nc.sync.dma_start(radii_o[:, cs], RD[:, cs])

    nc.compile()
    return nc


def _get_nc():
    global _NC_CACHE
    if _NC_CACHE is None:
        _NC_CACHE = _build_nc()
    return _NC_CACHE


def _make_in_maps(candidates, candidates_norms):
    cand2d = np.ascontiguousarray(candidates, dtype=np.float32).reshape(2048, RAW)
    nrm2d = np.ascontiguousarray(candidates_norms, dtype=np.float32).reshape(2048, RAW)

    ident = np.eye(R, dtype=np.float32)
    shift = np.zeros((R, R), dtype=np.float32)
    shift[np.arange(R), (np.arange(R) + 1) % R] = 1.0
    mats = np.concatenate([ident, shift], axis=1)

    in_maps = []
    for c in range(NCORES):
        r0 = c * R
        jrows = np.clip(np.arange(r0 - 1, r0 + R + 1), 0, NV - 1)
        cand_shard = np.ascontiguousarray(cand2d[2 * jrows])
        nrm_shard = np.ascontiguousarray(nrm2d[2 * np.arange(r0, r0 + R)])

        cv = np.zeros((R, 32), dtype=np.float32)
        jglob = np.arange(r0, r0 + R)
        rowmask = (jglob < NV - 1).astype(np.float32)
        cv[:, 0] = rowmask
        rs = np.ones(R, np.float32)
        rb = np.zeros(R, np.float32)
        if c == 0:
            rs[0] = 16.0 / 9.0
            rb[0] = -1.0 / 45.0
        if c == NCORES - 1:
            rs[-1] = 16.0 / 9.0
            rb[-1] = -1.0 / 45.0
        cv[:, 1] = rs
        cv[:, 2] = rb
        cv[:, 3] = 0.25 * rowmask
        cv[:, 4] = np.float32(2.5e-14) * rowmask
        cv[:, 5] = 0.0 if c == 0 else 1.0
        cv[:, 8:32] = 1.0   # LH boundary mask
        cv[0, 8] = 0.0      # L1 halo term invalid at i=0
        cv[127, 31] = 0.0   # L3 halo term invalid at i=1023

        in_maps.append(
            {"cand": cand_shard, "nrmr": nrm_shard, "cvec": cv, "mats": mats}
        )
    return in_maps


def kernel(valid, candidates, candidates_norms, step):
    global LAST_RESULT
    assert int(step) == 2, f"kernel hardcoded for step=2, got {step}"
    assert tuple(np.shape(valid)) == (2048, 2048)
    assert tuple(np.shape(candidates)) == (2048 * 2048, 3)

    from concourse.bass_utils import run_bass_kernel_spmd

    nc = _get_nc()
    in_maps = _make_in_maps(candidates, candidates_norms)
    res = run_bass_kernel_spmd(nc, in_maps, core_ids=list(range(NCORES)))
    LAST_RESULT = res

    N = NV * NU
    pts = np.empty((NV, NU, 3), np.float32)
    nrm = np.empty((NV, NU, 3), np.float32)
    radii = np.empty((NV, NU), np.float32)
    lens = np.empty((4, NV, NU), np.float32)
    areas = np.empty((2, NV, NU), np.float32)
    for c in range(NCORES):
        r0 = c * R
        r = res.results[c]
        pts[r0:r0 + R] = r["pts_o"].reshape(R, NU, 3)
        nrm[r0:r0 + R] = r["nrm_o"].reshape(R, NU, 3)
        radii[r0:r0 + R] = r["radii_o"].reshape(R, NU)
        lens[:, r0:r0 + R] = r["lens_o"].reshape(4, R, NU)
        areas[:, r0:r0 + R] = r["areas_o"].reshape(2, R, NU)

    return (
        pts.reshape(N, 3),
        nrm.reshape(N, 3),
        radii.reshape(N),
        lens.reshape(4 * N),
        areas.reshape(2 * N),
    )


# revision 37
# speedup vs baseline: 1.1266x; 1.0501x over previous
"""Trainium2 Bass kernel for nn_DirectDeformGraph (grid-mesh graph build).

Contract: kernel(**inputs) takes the FULL unsharded inputs from
setup_inputs() and returns the full outputs (pts, nrm, radii, lens, areas).

Strategy (hardcoded for H=W=2048, step=2 -> 1024x1024 node grid):
  - Shard the node grid by rows across 8 NeuronCores: 128 node-rows/core.
  - Each core receives only the even pixel rows it needs (its 128 rows plus
    one halo row above and below), does the column-subsample gather
    on-chip, and computes all outputs with pure shift-stencil math --
    the segment sums of the regular 4-edge stencil reduce to shifted adds,
    with the cross-shard (j-1) row handled by a tiny recomputed halo and a
    cyclic-shift matmul on the TensorEngine.
  - Areas use the Lagrange identity |AxB|^2 = |A|^2|B|^2 - (A.B)^2 to reuse
    the squared edge lengths already computed for `lens`.
"""

import os
import sys

import numpy as np

# concourse (Bass) ships with the container; make sure it resolves even when
# kernel.py is invoked from a bare directory.
try:
    import concourse  # noqa: F401
except ImportError:
    for p in ("/opt/trn_rl_repo", "/root/.axon_site/_ro/trn_rl_repo"):
        if os.path.isdir(p) and p not in sys.path:
            sys.path.insert(0, p)
    import concourse  # noqa: F401

NV = 1024          # node grid rows
NU = 1024          # node grid cols
R = 128            # node rows per core
NCORES = 8
F = NU * 3         # 3072 floats per compacted row
FP = F + 24        # padded tile width (pad cols are zeroed)
RAW = NU * 6       # 6144 floats per raw (full) pixel row
W_PIX = 2048

_NC_CACHE = None
LAST_RESULT = None


def _build_nc():
    import concourse.bacc as bacc
    import concourse.mybir as mybir
    from concourse.tile import TileContext

    f32 = mybir.dt.float32
    AF = mybir.ActivationFunctionType
    ALU = mybir.AluOpType

    nc = bacc.Bacc(trn_type="TRN2")

    cand = nc.dram_tensor("cand", [R + 2, RAW], f32, kind="ExternalInput")
    nrmr = nc.dram_tensor("nrmr", [R, RAW], f32, kind="ExternalInput")
    cvec = nc.dram_tensor("cvec", [R, 32], f32, kind="ExternalInput")
    mats = nc.dram_tensor("mats", [R, 256], f32, kind="ExternalInput")

    pts_o = nc.dram_tensor("pts_o", [R, F], f32, kind="ExternalOutput")
    nrm_o = nc.dram_tensor("nrm_o", [R, F], f32, kind="ExternalOutput")
    radii_o = nc.dram_tensor("radii_o", [R, NU], f32, kind="ExternalOutput")
    lens_o = nc.dram_tensor("lens_o", [4, R, NU], f32, kind="ExternalOutput")
    areas_o = nc.dram_tensor("areas_o", [2, R, NU], f32, kind="ExternalOutput")

    def even_cols(t):
        # [p, (i s c)] -> take s=0 (even pixel columns), keep 3 components
        return t.rearrange("p (i s c) -> p i s c", s=2, c=3)[:, :, 0, :]

    def tri(t):
        return t.rearrange("p (i c) -> p i c", c=3)

    with TileContext(nc) as tc:
        with (
            tc.tile_pool(name="pool", bufs=1) as pool,
            tc.tile_pool(name="psum", bufs=1, space="PSUM") as pp,
        ):
            # ---- load raw shards -------------------------------------------------
            # SP ring: halo-below row, rawA left half, P1 row-127 un-spread,
            # spread rows, pts/lens/radii out.
            # Pool ring: rawA right half, P1 shift, norms, nrm/areas out.
            HF = RAW // 2
            HC = F // 2
            rawA = pool.tile([R, RAW], f32, tag="raw24", bufs=2)  # node rows r0..r0+127
            cv = pool.tile([R, 32], f32)
            mm = pool.tile([R, 256], f32)
            # halo-below row via 16-way spread (64 nodes x 6 floats per
            # partition, contiguous chunks -> efficient descriptors)
            hb16 = pool.tile([16, 384], f32)
            nc.sync.dma_start(hb16[:, :],
                              cand[129, :].rearrange("(p x) -> p x", x=384))
            nc.sync.dma_start(rawA[:, 0:HF], cand[1:129, 0:HF])
            nc.gpsimd.dma_start(rawA[:, HF:RAW], cand[1:129, HF:RAW])
            nc.sync.dma_start(cv[:, :], cvec[:, :])
            nc.sync.dma_start(mm[:, :], mats[:, :])
            # 1/cnt pattern is constant -- build it up front so the radii
            # eviction never waits on it
            rc = pool.tile([R, NU], f32)
            nc.gpsimd.memset(rc[:, :], 0.125)
            nc.gpsimd.memset(rc[:, 0:1], 0.2)
            nc.gpsimd.memset(rc[:, NU - 1:NU], 0.2)
            nc.vector.tensor_scalar(rc[:, :], rc[:, :], cv[:, 1:2], cv[:, 2:3],
                                    ALU.mult, ALU.add)
            PHB = pool.tile([16, 192], f32)
            nc.scalar.copy(
                PHB.rearrange("p (i c) -> p i c", c=3),
                hb16.rearrange("p (i s c) -> p i s c", s=2, c=3)[:, :, 0, :],
            )

            # ---- compact even pixel columns (gather) on ACT ----------------------
            P0 = pool.tile([R, FP], f32)
            P1 = pool.tile([R, FP], f32)
            nc.vector.memset(P0[:, F:FP], 0.0)
            nc.vector.memset(P1[:, F:FP], 0.0)
            nc.scalar.copy(tri(P0[:, 0:HC]), even_cols(rawA[:, 0:HF]))
            nc.scalar.copy(tri(P0[:, HC:F]), even_cols(rawA[:, HF:RAW]))
            # P1 = P0 shifted down one row (SBUF->SBUF, no extra HBM read);
            # row 127 = un-spread of the compacted halo-below row.
            nc.sync.dma_start(
                P1[127:128, 0:F].rearrange("p (q k) -> p q k", k=192),
                PHB[:, :],
            )
            nc.gpsimd.dma_start(P1[0:127, 0:HC], P0[1:128, 0:HC])
            nc.gpsimd.dma_start(P1[0:127, HC:F], P0[1:128, HC:F])
            nc.sync.dma_start(pts_o[:, :], P0[:, 0:F])

            def spread_raw_row(drow):
                t = pool.tile([R, 60], f32, tag=f"hs{drow}")
                nc.vector.memset(t[0:1, 0:6], 0.0)
                nc.vector.memset(t[96:128, 54:60], 0.0)
                nc.sync.dma_start(t[0:1, 6:60], cand[drow:drow + 1, 0:54])
                nc.sync.dma_start(
                    t[1:128, 0:48],
                    cand[drow, 42:6138].rearrange("(p x) -> p x", x=48),
                )
                nc.sync.dma_start(
                    t[1:127, 48:60],
                    cand[drow, 90:6138].rearrange("(p x) -> p x", x=48)[:, 0:12],
                )
                nc.sync.dma_start(t[127:128, 48:54], cand[drow:drow + 1, 6138:6144])
                # compact even pixel columns -> [128, 30] (10 nodes x 3)
                c = pool.tile([R, 30], f32, tag=f"hc{drow}")
                nc.scalar.copy(
                    c.rearrange("p (i c) -> p i c", c=3),
                    t.rearrange("p (i s c) -> p i s c", s=2, c=3)[:, :, 0, :],
                )
                return c

            PH = spread_raw_row(0)     # node row r0-1, spread+compacted

            # norms: raw in (Pool), compact, out (Pool)
            nrw = pool.tile([R, RAW], f32, tag="raw24", bufs=2)
            nc.gpsimd.dma_start(nrw[:, 0:HF], nrmr[:, 0:HF])
            nc.gpsimd.dma_start(nrw[:, HF:RAW], nrmr[:, HF:RAW])
            nrc = pool.tile([R, F], f32)
            nc.scalar.copy(tri(nrc[:, 0:HC]), even_cols(nrw[:, 0:HF]))
            nc.scalar.copy(tri(nrc[:, HC:F]), even_cols(nrw[:, HF:RAW]))
            nc.gpsimd.dma_start(nrm_o[:, :], nrc[:, :])

            # spread the r0 row (cand row 1) the same way as the halo rows:
            # PB partition p holds nodes i in [8p-1, 8p+8] (10 nodes x 3).
            PB = spread_raw_row(1)

            # ---- halo W row: W(r0-1, i) = L1(r0-1,i-1) + L2(r0-1,i) + L3(r0-1,i)
            # term1 = |p(r0-1,i-1) - p(r0,i)|, term2 = |p(r0-1,i) - p(r0,i)|,
            # term3 = |p(r0-1,i+1) - p(r0,i)|, each 8 nodes per partition.
            DHL = pool.tile([R, 72], f32)
            nc.vector.tensor_sub(DHL[:, 0:24], PH[:, 0:24], PB[:, 3:27])
            nc.vector.tensor_sub(DHL[:, 24:48], PH[:, 3:27], PB[:, 3:27])
            nc.vector.tensor_sub(DHL[:, 48:72], PH[:, 6:30], PB[:, 3:27])
            SQH = pool.tile([R, 72], f32)
            nc.scalar.square(SQH[:, :], DHL[:, :])
            sqh3 = tri(SQH)
            TH = pool.tile([R, 24], f32)
            SSH = pool.tile([R, 24], f32)
            nc.vector.tensor_add(TH[:, :], sqh3[:, :, 0], sqh3[:, :, 1])
            nc.vector.tensor_add(SSH[:, :], TH[:, :], sqh3[:, :, 2])
            LH = pool.tile([R, 24], f32)
            nc.scalar.activation(LH[:, :], SSH[:, :], AF.Sqrt, scale=cv[:, 5:6])
            # zero the L1 term at i=0 and the L3 term at i=1023 (host-built mask)
            nc.vector.tensor_mul(LH[:, :], LH[:, :], cv[:, 8:32])
            TW = pool.tile([R, 8], f32)
            WH = pool.tile([R, 8], f32)
            nc.vector.tensor_add(TW[:, :], LH[:, 0:8], LH[:, 8:16])
            nc.vector.tensor_add(WH[:, :], TW[:, :], LH[:, 16:24])

            # ---- edge pipeline ---------------------------------------------------
            # Order chosen for overlap: D0 only needs P0 (starts earliest),
            # tri2's dot product runs mid-stream, edge 3 + radii form the tail.
            # L4 holds the four L tensors at stride NU+1 with a leading zero
            # column each, so shifted (i-1) matmul terms read [zero|data] and
            # keep a full-bank out AP.
            NP = NU + 1
            L4 = pool.tile([R, 4 * NP], f32)
            l4v = L4.rearrange("p (e i) -> p e i", i=NP)
            nc.vector.memset(l4v[:, :, 0:1], 0.0)

            D0 = pool.tile([R, F], f32)
            D1 = pool.tile([R, F], f32)
            D2 = pool.tile([R, F], f32)
            nc.vector.tensor_sub(D0[:, 0:HC - 3], P0[:, 0:HC - 3], P0[:, 3:HC])
            nc.vector.tensor_sub(D0[:, HC - 3:F], P0[:, HC - 3:F], P0[:, HC:F + 3])
            nc.vector.tensor_sub(D1[:, :], P0[:, 0:F], P1[:, 3:F + 3])
            nc.vector.tensor_sub(D2[:, :], P0[:, 0:F], P1[:, 0:F])

            SS = {}
            L = {}

            def edge(e, D):
                SQ = pool.tile([R, F], f32, tag="m12", bufs=2, name=f"SQ{e}")
                if e == 3:
                    # ACT is the gate at this point in the schedule; square the
                    # last edge on the (idle) GPSIMD instead
                    nc.gpsimd.tensor_mul(SQ[:, :], D[:, :], D[:, :])
                else:
                    nc.scalar.square(SQ[:, :], D[:, :])
                s3 = tri(SQ)
                T = pool.tile([R, NU], f32, tag="t4", bufs=4, name=f"T{e}")
                sstag = "t4" if e == 3 else f"ss{e}"
                SSe = pool.tile([R, NU + 1], f32, tag=sstag,
                                bufs=4 if e == 3 else 1, name=f"SS{e}")
                nc.vector.tensor_add(T[:, :], s3[:, :, 0], s3[:, :, 1])
                nc.vector.tensor_add(SSe[:, 0:NU], T[:, :], s3[:, :, 2])
                Le = L4[:, e * NP + 1:(e + 1) * NP]
                if e == 0:
                    nc.scalar.sqrt(Le, SSe[:, 0:NU])
                else:
                    # row mask (j < 1023) folded into the sqrt scale
                    nc.scalar.activation(Le, SSe[:, 0:NU], AF.Sqrt,
                                         scale=cv[:, 0:1])
                if e in (0, 1, 3):
                    # edges leaving i=1023 to the right don't exist
                    nc.gpsimd.memset(Le[:, NU - 1:NU], 0.0)
                nc.sync.dma_start(lens_o[e], Le)
                SS[e] = SSe
                L[e] = Le

            edge(0, D0)
            edge(1, D1)
            edge(2, D2)
            nc.vector.memset(SS[2][:, NU:NU + 1], 0.0)

            # edge 3 (D3 rotates through m12)
            D3 = pool.tile([R, F], f32, tag="m12", bufs=2, name="D3")
            nc.vector.tensor_sub(D3[:, :], P0[:, 3:F + 3], P1[:, 0:F])
            edge(3, D3)

            # tri2 dot product (needs D1, D2 -- runs mid-stream)
            M = pool.tile([R, F], f32, tag="m12", bufs=2)
            nc.vector.tensor_mul(M[:, :], D1[:, :], D2[:, :])
            m3 = tri(M)
            TD = pool.tile([R, NU], f32, tag="t4", bufs=4)
            DOT2 = pool.tile([R, NU], f32, tag="t4", bufs=4)
            nc.vector.tensor_add(TD[:, :], m3[:, :, 0], m3[:, :, 1])
            nc.vector.tensor_add(DOT2[:, :], TD[:, :], m3[:, :, 2])

            # ---- areas via Lagrange identity -------------------------------------
            # |DaxDb|^2 = SSa*SSb - dot^2; for tri1,
            # dot1 = (SS0 + SS1 - |D0-D1|^2)/2 and D0-D1 = -D2(i+1), so
            # dot1 = (SS0 + SS1 - SS2(i+1))/2 -- no product tensor needed.
            def area_tail(t, DOT, qscale, ea, eb):
                Q = pool.tile([R, NU], f32, tag="t4", bufs=4, name=f"Q{t}")
                nc.scalar.activation(Q[:, :], DOT[:, :], AF.Square, scale=qscale)
                TT = pool.tile([R, NU], f32, tag="t4", bufs=4, name=f"TT{t}")
                nc.gpsimd.tensor_mul(TT[:, :], SS[ea][:, 0:NU], SS[eb][:, 0:NU])
                S = pool.tile([R, NU], f32, tag="t4", bufs=4, name=f"S{t}")
                nc.gpsimd.tensor_sub(S[:, :], TT[:, :], Q[:, :])
                nc.vector.tensor_scalar_max(S[:, :], S[:, :], 0.0)
                A = pool.tile([R, NU], f32, tag="t4", bufs=4, name=f"A{t}")
                nc.scalar.activation(A, S[:, :], AF.Sqrt,
                                     scale=cv[:, 3:4], bias=cv[:, 4:5])
                nc.gpsimd.memset(A[:, NU - 1:NU], 0.0)
                (nc.sync if t == 0 else nc.gpsimd).dma_start(areas_o[t], A[:, :])

            X = pool.tile([R, NU], f32, tag="t4", bufs=4)
            nc.vector.tensor_add(X[:, :], SS[0][:, 0:NU], SS[1][:, 0:NU])
            DOT1 = pool.tile([R, NU], f32, tag="t4", bufs=4)
            nc.vector.tensor_sub(DOT1[:, :], X[:, :], SS[2][:, 1:NU + 1])
            area_tail(0, DOT1, 0.5, 0, 1)
            area_tail(1, DOT2, 1.0, 1, 2)

            # ---- radii: full stencil on PE (float32r, 1 cyc/row) ------------------
            # ssum = L0+L1+L2 + (L3+L0)(i-1) + shift_down(W),
            # W = L1(i-1)+L2+L3 with the halo row injected at partition 127
            # and rotated in by the cyclic shift matrix SC.
            Wt = pool.tile([R, NU], f32)
            # halo W row lands in partition 127 early (off the critical path);
            # the adds below only touch partitions 0..126 (the rows the cyclic
            # shift consumes alongside row 127).
            nc.sync.dma_start(
                Wt[127:128, :].rearrange("p (q k) -> p q k", k=8), WH[:, :]
            )
            nc.gpsimd.tensor_add(Wt[0:127, :], L[2][0:127, :], L[3][0:127, :])
            nc.gpsimd.tensor_add(Wt[0:127, 1:NU], Wt[0:127, 1:NU],
                                 L[1][0:127, 0:NU - 1])

            # U = L0+L1+L2+(L3+L0)(i-1) on DVE; PE adds shift_down(W) only
            # (longer fp32 matmul accumulation chains crashed on HW)
            U = pool.tile([R, NU], f32)
            nc.gpsimd.tensor_add(U[:, :], L[0], L[1])
            nc.gpsimd.tensor_add(U[:, :], U[:, :], L[2])
            nc.gpsimd.tensor_add(U[:, 1:NU], U[:, 1:NU], L[3][:, 0:NU - 1])
            nc.gpsimd.tensor_add(U[:, 1:NU], U[:, 1:NU], L[0][:, 0:NU - 1])

            ps = pp.tile([R, NU], f32)
            SC = mm[:, 128:256]
            for h in range(2):
                cs = slice(512 * h, 512 * h + 512)
                nc.tensor.matmul(ps[:, cs], SC, Wt[:, cs], start=True,
                                 stop=True)

            # ssum = U + shift_down(W); two DVE ops per half so the radii DMA
            # drains as soon as each bank closes
            RD = pool.tile([R, NU], f32, tag="t4", bufs=4)
            for h in range(2):
                cs = slice(512 * h, 512 * h + 512)
                TMP = pool.tile([R, 512], f32, tag="t4", bufs=4, name=f"TMP{h}")
                nc.vector.tensor_add(TMP[:, :], ps[:, cs], U[:, cs])
                nc.vector.tensor_mul(RD[:, cs], TMP[:, :], rc[:, cs])
                nc.sync.dma_start(radii_o[:, cs], RD[:, cs])

    nc.compile()
    return nc


def _get_nc():
    global _NC_CACHE
    if _NC_CACHE is None:
        _NC_CACHE = _build_nc()
    return _NC_CACHE


def _make_in_maps(candidates, candidates_norms):
    cand2d = np.ascontiguousarray(candidates, dtype=np.float32).reshape(2048, RAW)
    nrm2d = np.ascontiguousarray(candidates_norms, dtype=np.float32).reshape(2048, RAW)

    ident = np.eye(R, dtype=np.float32)
    shift = np.zeros((R, R), dtype=np.float32)
    shift[np.arange(R), (np.arange(R) + 1) % R] = 1.0
    mats = np.concatenate([ident, shift], axis=1)

    in_maps = []
    for c in range(NCORES):
        r0 = c * R
        jrows = np.clip(np.arange(r0 - 1, r0 + R + 1), 0, NV - 1)
        cand_shard = np.ascontiguousarray(cand2d[2 * jrows])
        nrm_shard = np.ascontiguousarray(nrm2d[2 * np.arange(r0, r0 + R)])

        cv = np.zeros((R, 32), dtype=np.float32)
        jglob = np.arange(r0, r0 + R)
        rowmask = (jglob < NV - 1).astype(np.float32)
        cv[:, 0] = rowmask
        rs = np.ones(R, np.float32)
        rb = np.zeros(R, np.float32)
        if c == 0:
            rs[0] = 16.0 / 9.0
            rb[0] = -1.0 / 45.0
        if c == NCORES - 1:
            rs[-1] = 16.0 / 9.0
            rb[-1] = -1.0 / 45.0
        cv[:, 1] = rs
        cv[:, 2] = rb
        cv[:, 3] = 0.25 * rowmask
        cv[:, 4] = np.float32(2.5e-14) * rowmask
        cv[:, 5] = 0.0 if c == 0 else 1.0
        cv[:, 8:32] = 1.0   # LH boundary mask
        cv[0, 8] = 0.0      # L1 halo term invalid at i=0
        cv[127, 31] = 0.0   # L3 halo term invalid at i=1023

        in_maps.append(
            {"cand": cand_shard, "nrmr": nrm_shard, "cvec": cv, "mats": mats}
        )
    return in_maps


def kernel(valid, candidates, candidates_norms, step):
    global LAST_RESULT
    assert int(step) == 2, f"kernel hardcoded for step=2, got {step}"
    assert tuple(np.shape(valid)) == (2048, 2048)
    assert tuple(np.shape(candidates)) == (2048 * 2048, 3)

    from concourse.bass_utils import run_bass_kernel_spmd

    nc = _get_nc()
    in_maps = _make_in_maps(candidates, candidates_norms)
    res = run_bass_kernel_spmd(nc, in_maps, core_ids=list(range(NCORES)))
    LAST_RESULT = res

    N = NV * NU
    pts = np.empty((NV, NU, 3), np.float32)
    nrm = np.empty((NV, NU, 3), np.float32)
    radii = np.empty((NV, NU), np.float32)
    lens = np.empty((4, NV, NU), np.float32)
    areas = np.empty((2, NV, NU), np.float32)
    for c in range(NCORES):
        r0 = c * R
        r = res.results[c]
        pts[r0:r0 + R] = r["pts_o"].reshape(R, NU, 3)
        nrm[r0:r0 + R] = r["nrm_o"].reshape(R, NU, 3)
        radii[r0:r0 + R] = r["radii_o"].reshape(R, NU)
        lens[:, r0:r0 + R] = r["lens_o"].reshape(4, R, NU)
        areas[:, r0:r0 + R] = r["areas_o"].reshape(2, R, NU)

    return (
        pts.reshape(N, 3),
        nrm.reshape(N, 3),
        radii.reshape(N),
        lens.reshape(4 * N),
        areas.reshape(2 * N),
    )


# revision 38
# speedup vs baseline: 1.1775x; 1.0452x over previous
"""Trainium2 Bass kernel for nn_DirectDeformGraph (grid-mesh graph build).

Contract: kernel(**inputs) takes the FULL unsharded inputs from
setup_inputs() and returns the full outputs (pts, nrm, radii, lens, areas).

Strategy (hardcoded for H=W=2048, step=2 -> 1024x1024 node grid):
  - Shard the node grid by rows across 8 NeuronCores: 128 node-rows/core.
  - Each core receives only the even pixel rows it needs (its 128 rows plus
    one halo row above and below), does the column-subsample gather
    on-chip, and computes all outputs with pure shift-stencil math --
    the segment sums of the regular 4-edge stencil reduce to shifted adds,
    with the cross-shard (j-1) row handled by a tiny recomputed halo and a
    cyclic-shift matmul on the TensorEngine.
  - Areas use the Lagrange identity |AxB|^2 = |A|^2|B|^2 - (A.B)^2 to reuse
    the squared edge lengths already computed for `lens`.
"""

import os
import sys

import numpy as np

# concourse (Bass) ships with the container; make sure it resolves even when
# kernel.py is invoked from a bare directory.
try:
    import concourse  # noqa: F401
except ImportError:
    for p in ("/opt/trn_rl_repo", "/root/.axon_site/_ro/trn_rl_repo"):
        if os.path.isdir(p) and p not in sys.path:
            sys.path.insert(0, p)
    import concourse  # noqa: F401

NV = 1024          # node grid rows
NU = 1024          # node grid cols
R = 128            # node rows per core
NCORES = 8
F = NU * 3         # 3072 floats per compacted row
FP = F + 24        # padded tile width (pad cols are zeroed)
RAW = NU * 6       # 6144 floats per raw (full) pixel row
W_PIX = 2048

_NC_CACHE = None
LAST_RESULT = None


def _build_nc():
    import concourse.bacc as bacc
    import concourse.mybir as mybir
    from concourse.tile import TileContext

    f32 = mybir.dt.float32
    AF = mybir.ActivationFunctionType
    ALU = mybir.AluOpType

    nc = bacc.Bacc(trn_type="TRN2")

    cand = nc.dram_tensor("cand", [R + 2, RAW], f32, kind="ExternalInput")
    nrmr = nc.dram_tensor("nrmr", [R, RAW], f32, kind="ExternalInput")
    cvec = nc.dram_tensor("cvec", [R, 32], f32, kind="ExternalInput")
    mats = nc.dram_tensor("mats", [R, 256], f32, kind="ExternalInput")

    pts_o = nc.dram_tensor("pts_o", [R, F], f32, kind="ExternalOutput")
    nrm_o = nc.dram_tensor("nrm_o", [R, F], f32, kind="ExternalOutput")
    radii_o = nc.dram_tensor("radii_o", [R, NU], f32, kind="ExternalOutput")
    lens_o = nc.dram_tensor("lens_o", [4, R, NU], f32, kind="ExternalOutput")
    areas_o = nc.dram_tensor("areas_o", [2, R, NU], f32, kind="ExternalOutput")

    def even_cols(t):
        # [p, (i s c)] -> take s=0 (even pixel columns), keep 3 components
        return t.rearrange("p (i s c) -> p i s c", s=2, c=3)[:, :, 0, :]

    def tri(t):
        return t.rearrange("p (i c) -> p i c", c=3)

    with TileContext(nc) as tc:
        with (
            tc.tile_pool(name="pool", bufs=1) as pool,
            tc.tile_pool(name="psum", bufs=1, space="PSUM") as pp,
        ):
            # ---- load raw shards -------------------------------------------------
            # SP ring: halo-below row, rawA left half, P1 row-127 un-spread,
            # spread rows, pts/lens/radii out.
            # Pool ring: rawA right half, P1 shift, norms, nrm/areas out.
            HF = RAW // 2
            HC = F // 2
            rawA = pool.tile([R, RAW], f32, tag="raw24", bufs=2)  # node rows r0..r0+127
            cv = pool.tile([R, 32], f32)
            mm = pool.tile([R, 256], f32)
            # halo-below row via 16-way spread (64 nodes x 6 floats per
            # partition, contiguous chunks -> efficient descriptors)
            hb16 = pool.tile([16, 384], f32)
            nc.sync.dma_start(hb16[:, :],
                              cand[129, :].rearrange("(p x) -> p x", x=384))
            nc.sync.dma_start(rawA[:, 0:HF], cand[1:129, 0:HF])
            nc.gpsimd.dma_start(rawA[:, HF:RAW], cand[1:129, HF:RAW])
            nc.sync.dma_start(cv[:, :], cvec[:, :])
            nc.sync.dma_start(mm[:, :], mats[:, :])
            # 1/cnt pattern is constant -- build it up front so the radii
            # eviction never waits on it
            rc = pool.tile([R, NU], f32)
            nc.gpsimd.memset(rc[:, :], 0.125)
            nc.gpsimd.memset(rc[:, 0:1], 0.2)
            nc.gpsimd.memset(rc[:, NU - 1:NU], 0.2)
            nc.vector.tensor_scalar(rc[:, :], rc[:, :], cv[:, 1:2], cv[:, 2:3],
                                    ALU.mult, ALU.add)
            PHB = pool.tile([16, 192], f32)
            nc.scalar.copy(
                PHB.rearrange("p (i c) -> p i c", c=3),
                hb16.rearrange("p (i s c) -> p i s c", s=2, c=3)[:, :, 0, :],
            )

            # ---- compact even pixel columns (gather) on ACT ----------------------
            P0 = pool.tile([R, FP], f32)
            P1 = pool.tile([R, FP], f32)
            nc.vector.memset(P0[:, F:FP], 0.0)
            nc.vector.memset(P1[:, F:FP], 0.0)
            nc.scalar.copy(tri(P0[:, 0:HC]), even_cols(rawA[:, 0:HF]))
            nc.scalar.copy(tri(P0[:, HC:F]), even_cols(rawA[:, HF:RAW]))
            # P1 = P0 shifted down one row (SBUF->SBUF, no extra HBM read);
            # row 127 = un-spread of the compacted halo-below row.
            nc.sync.dma_start(
                P1[127:128, 0:F].rearrange("p (q k) -> p q k", k=192),
                PHB[:, :],
            )
            nc.gpsimd.dma_start(P1[0:127, 0:HC], P0[1:128, 0:HC])
            nc.gpsimd.dma_start(P1[0:127, HC:F], P0[1:128, HC:F])
            nc.sync.dma_start(pts_o[:, :], P0[:, 0:F])

            def spread_raw_row(drow):
                t = pool.tile([R, 60], f32, tag=f"hs{drow}")
                nc.vector.memset(t[0:1, 0:6], 0.0)
                nc.vector.memset(t[96:128, 54:60], 0.0)
                nc.sync.dma_start(t[0:1, 6:60], cand[drow:drow + 1, 0:54])
                nc.sync.dma_start(
                    t[1:128, 0:48],
                    cand[drow, 42:6138].rearrange("(p x) -> p x", x=48),
                )
                nc.sync.dma_start(
                    t[1:127, 48:60],
                    cand[drow, 90:6138].rearrange("(p x) -> p x", x=48)[:, 0:12],
                )
                nc.sync.dma_start(t[127:128, 48:54], cand[drow:drow + 1, 6138:6144])
                # compact even pixel columns -> [128, 30] (10 nodes x 3)
                c = pool.tile([R, 30], f32, tag=f"hc{drow}")
                nc.scalar.copy(
                    c.rearrange("p (i c) -> p i c", c=3),
                    t.rearrange("p (i s c) -> p i s c", s=2, c=3)[:, :, 0, :],
                )
                return c

            PH = spread_raw_row(0)     # node row r0-1, spread+compacted

            # norms: raw in (Pool), compact, out (Pool)
            nrw = pool.tile([R, RAW], f32, tag="raw24", bufs=2)
            nc.gpsimd.dma_start(nrw[:, 0:HF], nrmr[:, 0:HF])
            nc.gpsimd.dma_start(nrw[:, HF:RAW], nrmr[:, HF:RAW])
            nrc = pool.tile([R, F], f32, tag="m12", bufs=3)
            nc.scalar.copy(tri(nrc[:, 0:HC]), even_cols(nrw[:, 0:HF]))
            nc.scalar.copy(tri(nrc[:, HC:F]), even_cols(nrw[:, HF:RAW]))
            nc.gpsimd.dma_start(nrm_o[:, :], nrc[:, :])

            # spread the r0 row (cand row 1) the same way as the halo rows:
            # PB partition p holds nodes i in [8p-1, 8p+8] (10 nodes x 3).
            PB = spread_raw_row(1)

            # ---- halo W row: W(r0-1, i) = L1(r0-1,i-1) + L2(r0-1,i) + L3(r0-1,i)
            # term1 = |p(r0-1,i-1) - p(r0,i)|, term2 = |p(r0-1,i) - p(r0,i)|,
            # term3 = |p(r0-1,i+1) - p(r0,i)|, each 8 nodes per partition.
            DHL = pool.tile([R, 72], f32)
            nc.vector.tensor_sub(DHL[:, 0:24], PH[:, 0:24], PB[:, 3:27])
            nc.vector.tensor_sub(DHL[:, 24:48], PH[:, 3:27], PB[:, 3:27])
            nc.vector.tensor_sub(DHL[:, 48:72], PH[:, 6:30], PB[:, 3:27])
            SQH = pool.tile([R, 72], f32)
            nc.scalar.square(SQH[:, :], DHL[:, :])
            sqh3 = tri(SQH)
            TH = pool.tile([R, 24], f32)
            SSH = pool.tile([R, 24], f32)
            nc.vector.tensor_add(TH[:, :], sqh3[:, :, 0], sqh3[:, :, 1])
            nc.vector.tensor_add(SSH[:, :], TH[:, :], sqh3[:, :, 2])
            LH = pool.tile([R, 24], f32)
            nc.scalar.activation(LH[:, :], SSH[:, :], AF.Sqrt, scale=cv[:, 5:6])
            # zero the L1 term at i=0 and the L3 term at i=1023 (host-built mask)
            nc.vector.tensor_mul(LH[:, :], LH[:, :], cv[:, 8:32])
            TW = pool.tile([R, 8], f32)
            WH = pool.tile([R, 8], f32)
            nc.vector.tensor_add(TW[:, :], LH[:, 0:8], LH[:, 8:16])
            nc.vector.tensor_add(WH[:, :], TW[:, :], LH[:, 16:24])

            # ---- edge pipeline ---------------------------------------------------
            # Order chosen for overlap: D0 only needs P0 (starts earliest),
            # tri2's dot product runs mid-stream, edge 3 + radii form the tail.
            # L4 holds the four L tensors at stride NU+1 with a leading zero
            # column each, so shifted (i-1) matmul terms read [zero|data] and
            # keep a full-bank out AP.
            NP = NU + 1
            L4 = pool.tile([R, 4 * NP], f32)
            l4v = L4.rearrange("p (e i) -> p e i", i=NP)
            nc.vector.memset(l4v[:, :, 0:1], 0.0)

            D0 = pool.tile([R, F], f32)
            D1 = pool.tile([R, F], f32)
            D2 = pool.tile([R, F], f32)
            nc.vector.tensor_sub(D0[:, 0:HC - 3], P0[:, 0:HC - 3], P0[:, 3:HC])
            nc.vector.tensor_sub(D0[:, HC - 3:F], P0[:, HC - 3:F], P0[:, HC:F + 3])
            nc.vector.tensor_sub(D1[:, :], P0[:, 0:F], P1[:, 3:F + 3])
            nc.vector.tensor_sub(D2[:, :], P0[:, 0:F], P1[:, 0:F])

            SS = {}
            L = {}

            def edge(e, D):
                SQ = pool.tile([R, F], f32, tag="m12", bufs=3, name=f"SQ{e}")
                if e == 3:
                    # ACT is the gate at this point in the schedule; square the
                    # last edge on the (idle) GPSIMD instead
                    nc.gpsimd.tensor_mul(SQ[:, :], D[:, :], D[:, :])
                else:
                    nc.scalar.square(SQ[:, :], D[:, :])
                s3 = tri(SQ)
                T = pool.tile([R, NU], f32, tag="t4", bufs=4, name=f"T{e}")
                sstag = "t4" if e == 3 else f"ss{e}"
                SSe = pool.tile([R, NU + 1], f32, tag=sstag,
                                bufs=4 if e == 3 else 1, name=f"SS{e}")
                nc.vector.tensor_add(T[:, :], s3[:, :, 0], s3[:, :, 1])
                nc.vector.tensor_add(SSe[:, 0:NU], T[:, :], s3[:, :, 2])
                Le = L4[:, e * NP + 1:(e + 1) * NP]
                if e == 0:
                    nc.scalar.sqrt(Le, SSe[:, 0:NU])
                else:
                    # row mask (j < 1023) folded into the sqrt scale
                    nc.scalar.activation(Le, SSe[:, 0:NU], AF.Sqrt,
                                         scale=cv[:, 0:1])
                if e in (0, 1, 3):
                    # edges leaving i=1023 to the right don't exist
                    nc.gpsimd.memset(Le[:, NU - 1:NU], 0.0)
                nc.sync.dma_start(lens_o[e], Le)
                SS[e] = SSe
                L[e] = Le

            edge(0, D0)
            edge(1, D1)
            edge(2, D2)
            nc.vector.memset(SS[2][:, NU:NU + 1], 0.0)

            # edge 3 (D3 rotates through m12)
            D3 = pool.tile([R, F], f32, tag="m12", bufs=3, name="D3")
            nc.vector.tensor_sub(D3[:, :], P0[:, 3:F + 3], P1[:, 0:F])
            edge(3, D3)

            # tri2 dot product (needs D1, D2 -- runs mid-stream)
            M = pool.tile([R, F], f32, tag="m12", bufs=3)
            nc.vector.tensor_mul(M[:, :], D1[:, :], D2[:, :])
            m3 = tri(M)
            TD = pool.tile([R, NU], f32, tag="t4", bufs=4)
            DOT2 = pool.tile([R, NU], f32, tag="t4", bufs=4)
            nc.vector.tensor_add(TD[:, :], m3[:, :, 0], m3[:, :, 1])
            nc.vector.tensor_add(DOT2[:, :], TD[:, :], m3[:, :, 2])

            # ---- areas via Lagrange identity -------------------------------------
            # |DaxDb|^2 = SSa*SSb - dot^2; for tri1,
            # dot1 = (SS0 + SS1 - |D0-D1|^2)/2 and D0-D1 = -D2(i+1), so
            # dot1 = (SS0 + SS1 - SS2(i+1))/2 -- no product tensor needed.
            def area_tail(t, DOT, qscale, ea, eb):
                Q = pool.tile([R, NU], f32, tag="t4", bufs=4, name=f"Q{t}")
                nc.scalar.activation(Q[:, :], DOT[:, :], AF.Square, scale=qscale)
                TT = pool.tile([R, NU], f32, tag="t4", bufs=4, name=f"TT{t}")
                nc.gpsimd.tensor_mul(TT[:, :], SS[ea][:, 0:NU], SS[eb][:, 0:NU])
                S = pool.tile([R, NU], f32, tag="t4", bufs=4, name=f"S{t}")
                nc.gpsimd.tensor_sub(S[:, :], TT[:, :], Q[:, :])
                nc.vector.tensor_scalar_max(S[:, :], S[:, :], 0.0)
                A = pool.tile([R, NU], f32, tag="t4", bufs=4, name=f"A{t}")
                nc.scalar.activation(A, S[:, :], AF.Sqrt,
                                     scale=cv[:, 3:4], bias=cv[:, 4:5])
                nc.gpsimd.memset(A[:, NU - 1:NU], 0.0)
                (nc.sync if t == 0 else nc.gpsimd).dma_start(areas_o[t], A[:, :])

            X = pool.tile([R, NU], f32, tag="t4", bufs=4)
            nc.vector.tensor_add(X[:, :], SS[0][:, 0:NU], SS[1][:, 0:NU])
            DOT1 = pool.tile([R, NU], f32, tag="t4", bufs=4)
            nc.vector.tensor_sub(DOT1[:, :], X[:, :], SS[2][:, 1:NU + 1])
            area_tail(0, DOT1, 0.5, 0, 1)
            area_tail(1, DOT2, 1.0, 1, 2)

            # ---- radii: full stencil on PE (float32r, 1 cyc/row) ------------------
            # ssum = L0+L1+L2 + (L3+L0)(i-1) + shift_down(W),
            # W = L1(i-1)+L2+L3 with the halo row injected at partition 127
            # and rotated in by the cyclic shift matrix SC.
            Wt = pool.tile([R, NU], f32)
            # halo W row lands in partition 127 early (off the critical path);
            # the adds below only touch partitions 0..126 (the rows the cyclic
            # shift consumes alongside row 127).
            nc.sync.dma_start(
                Wt[127:128, :].rearrange("p (q k) -> p q k", k=8), WH[:, :]
            )
            nc.gpsimd.tensor_add(Wt[0:127, :], L[2][0:127, :], L[3][0:127, :])
            nc.gpsimd.tensor_add(Wt[0:127, 1:NU], Wt[0:127, 1:NU],
                                 L[1][0:127, 0:NU - 1])

            # U = L0+L1+L2+(L3+L0)(i-1) on DVE; PE adds shift_down(W) only
            # (longer fp32 matmul accumulation chains crashed on HW)
            U = pool.tile([R, NU], f32)
            nc.gpsimd.tensor_add(U[:, :], L[0], L[1])
            nc.gpsimd.tensor_add(U[:, :], U[:, :], L[2])
            nc.gpsimd.tensor_add(U[:, 1:NU], U[:, 1:NU], L[3][:, 0:NU - 1])
            nc.gpsimd.tensor_add(U[:, 1:NU], U[:, 1:NU], L[0][:, 0:NU - 1])

            ps = pp.tile([R, NU], f32)
            SC = mm[:, 128:256]
            for h in range(2):
                cs = slice(512 * h, 512 * h + 512)
                nc.tensor.matmul(ps[:, cs], SC, Wt[:, cs], start=True,
                                 stop=True)

            # ssum = U + shift_down(W); two DVE ops per half so the radii DMA
            # drains as soon as each bank closes
            RD = pool.tile([R, NU], f32, tag="t4", bufs=4)
            for h in range(2):
                cs = slice(512 * h, 512 * h + 512)
                TMP = pool.tile([R, 512], f32, tag="t4", bufs=4, name=f"TMP{h}")
                nc.vector.tensor_add(TMP[:, :], ps[:, cs], U[:, cs])
                nc.vector.tensor_mul(RD[:, cs], TMP[:, :], rc[:, cs])
                nc.sync.dma_start(radii_o[:, cs], RD[:, cs])

    nc.compile()
    return nc


def _get_nc():
    global _NC_CACHE
    if _NC_CACHE is None:
        _NC_CACHE = _build_nc()
    return _NC_CACHE


def _make_in_maps(candidates, candidates_norms):
    cand2d = np.ascontiguousarray(candidates, dtype=np.float32).reshape(2048, RAW)
    nrm2d = np.ascontiguousarray(candidates_norms, dtype=np.float32).reshape(2048, RAW)

    ident = np.eye(R, dtype=np.float32)
    shift = np.zeros((R, R), dtype=np.float32)
    shift[np.arange(R), (np.arange(R) + 1) % R] = 1.0
    mats = np.concatenate([ident, shift], axis=1)

    in_maps = []
    for c in range(NCORES):
        r0 = c * R
        jrows = np.clip(np.arange(r0 - 1, r0 + R + 1), 0, NV - 1)
        cand_shard = np.ascontiguousarray(cand2d[2 * jrows])
        nrm_shard = np.ascontiguousarray(nrm2d[2 * np.arange(r0, r0 + R)])

        cv = np.zeros((R, 32), dtype=np.float32)
        jglob = np.arange(r0, r0 + R)
        rowmask = (jglob < NV - 1).astype(np.float32)
        cv[:, 0] = rowmask
        rs = np.ones(R, np.float32)
        rb = np.zeros(R, np.float32)
        if c == 0:
            rs[0] = 16.0 / 9.0
            rb[0] = -1.0 / 45.0
        if c == NCORES - 1:
            rs[-1] = 16.0 / 9.0
            rb[-1] = -1.0 / 45.0
        cv[:, 1] = rs
        cv[:, 2] = rb
        cv[:, 3] = 0.25 * rowmask
        cv[:, 4] = np.float32(2.5e-14) * rowmask
        cv[:, 5] = 0.0 if c == 0 else 1.0
        cv[:, 8:32] = 1.0   # LH boundary mask
        cv[0, 8] = 0.0      # L1 halo term invalid at i=0
        cv[127, 31] = 0.0   # L3 halo term invalid at i=1023

        in_maps.append(
            {"cand": cand_shard, "nrmr": nrm_shard, "cvec": cv, "mats": mats}
        )
    return in_maps


def kernel(valid, candidates, candidates_norms, step):
    global LAST_RESULT
    assert int(step) == 2, f"kernel hardcoded for step=2, got {step}"
    assert tuple(np.shape(valid)) == (2048, 2048)
    assert tuple(np.shape(candidates)) == (2048 * 2048, 3)

    from concourse.bass_utils import run_bass_kernel_spmd

    nc = _get_nc()
    in_maps = _make_in_maps(candidates, candidates_norms)
    res = run_bass_kernel_spmd(nc, in_maps, core_ids=list(range(NCORES)))
    LAST_RESULT = res

    N = NV * NU
    pts = np.empty((NV, NU, 3), np.float32)
    nrm = np.empty((NV, NU, 3), np.float32)
    radii = np.empty((NV, NU), np.float32)
    lens = np.empty((4, NV, NU), np.float32)
    areas = np.empty((2, NV, NU), np.float32)
    for c in range(NCORES):
        r0 = c * R
        r = res.results[c]
        pts[r0:r0 + R] = r["pts_o"].reshape(R, NU, 3)
        nrm[r0:r0 + R] = r["nrm_o"].reshape(R, NU, 3)
        radii[r0:r0 + R] = r["radii_o"].reshape(R, NU)
        lens[:, r0:r0 + R] = r["lens_o"].reshape(4, R, NU)
        areas[:, r0:r0 + R] = r["areas_o"].reshape(2, R, NU)

    return (
        pts.reshape(N, 3),
        nrm.reshape(N, 3),
        radii.reshape(N),
        lens.reshape(4 * N),
        areas.reshape(2 * N),
    )
